# revision 1
# baseline (speedup 1.0000x reference)
"""Multi-head attention (B=4, S=2048, D=1024, H=16) on 8 TRN2 NeuronCores.

Sharding (Megatron-style, per spec hint): data-parallel over batch (4) x
tensor-parallel over heads (2 groups of 8). Core c handles batch c//2,
head-group c%2. QKV projections column-sharded, output projection
row-sharded; the two partial outputs per batch are summed on the host
together with the output bias.

Per-core kernel (one NeuronCore, 8 heads, 2048 tokens):
  - Host passes x pre-transposed (xT [D, S]) so projection matmuls can
    contract over D on partitions without any on-chip transposes.
  - k is projected feature-major (kT [512, S]); v token-major into an
    ones-augmented layout (v_aug [k, 65] per (k-tile, head), bf16) so the
    att@V matmul yields both the attention output and the softmax
    denominator Z in one stream of the probabilities.
  - Scores are computed transposed, ST[k, q] = (K Q^T); softmax skips
    max-subtraction (logits are ~N(0,1), safe for fp32 exp) so exp is one
    ACT pass per score tile with the 1/sqrt(dk) folded into ACT's scale,
    written as bf16.
  - Pipeline: only the k projection runs as a prelude; the v projection is
    spliced into the first attention pair's score loop, q-projection
    chains and the previous group's output-projection chains are spliced
    between attention pairs, so ScalarE (exp, the bottleneck engine)
    starts early and streams with few gaps.
  - Matmuls run as float32r (TF32, full rate at N=512) except att@V
    (bf16 probabilities / values).
"""

import sys

if "/opt/trn_rl_repo" not in sys.path:
    sys.path.insert(0, "/opt/trn_rl_repo")

import numpy as np

B, S, D = 4, 2048, 1024
H, DK = 16, 64
NCORES = 8
HC = H // 2            # heads per core
DC = HC * DK           # 512 local features per core
INV_SCALE = 1.0 / 8.0  # 1/sqrt(DK)
P = 128
NDCH = D // P          # 8 contraction chunks for projections
NFC = DC // P          # 4 local feature chunks
NKT = S // P           # 16 key tiles
NQG = 4                # query groups
QG = S // NQG          # 512 queries per group
VW = DK + 1            # 65: v columns + ones column
NHP = HC // 2          # head pairs

_CACHE = {}


def _build():
    import concourse.bass as bass
    import concourse.bacc as bacc
    import concourse.tile as tile
    import concourse.mybir as mybir
    from concourse.bass import ts, ds

    f32 = mybir.dt.float32
    f32r = mybir.dt.float32r
    bf16 = mybir.dt.bfloat16
    AF = mybir.ActivationFunctionType
    ALU = mybir.AluOpType

    nc = bacc.Bacc("TRN2", target_bir_lowering=False, num_devices=NCORES)

    xqT = nc.dram_tensor("xqT", [D, S], bf16, kind="ExternalInput")
    xkT = nc.dram_tensor("xkT", [D, S], bf16, kind="ExternalInput")
    xvT = nc.dram_tensor("xvT", [D, S], bf16, kind="ExternalInput")
    wq = nc.dram_tensor("wq", [D, DC], bf16, kind="ExternalInput")
    wk = nc.dram_tensor("wk", [D, DC], bf16, kind="ExternalInput")
    wv = nc.dram_tensor("wv", [D, DC], bf16, kind="ExternalInput")
    wo = nc.dram_tensor("wo", [DC, D], f32r, kind="ExternalInput")
    bq = nc.dram_tensor("bq", [DC], f32, kind="ExternalInput")
    bk = nc.dram_tensor("bk", [DC], f32, kind="ExternalInput")
    bv = nc.dram_tensor("bv", [DC], f32, kind="ExternalInput")
    out = nc.dram_tensor("out", [S, D], f32, kind="ExternalOutput")

    with tile.TileContext(nc) as tc:
        with (
            tc.tile_pool(name="persist", bufs=1) as persist,
            tc.tile_pool(name="wts", bufs=2) as wpool,
            tc.tile_pool(name="xin", bufs=3) as xpool,
            tc.tile_pool(name="qt", bufs=2) as qpool,
            tc.tile_pool(name="expst", bufs=18) as epool,
            tc.tile_pool(name="outt", bufs=2) as opool,
            tc.tile_pool(name="small", bufs=2) as spool,
            tc.tile_pool(name="osb", bufs=4) as osb_pool,
            tc.tile_pool(name="misc", bufs=2, space="PSUM") as pp,
            tc.tile_pool(name="st", bufs=2, space="PSUM") as st_pool,
            tc.tile_pool(name="av", bufs=2, space="PSUM") as avp,
        ):
            # ---- persistent SBUF tensors ----
            kT = persist.tile([P, NFC, S], bf16)          # 16KB/part
            v_aug = persist.tile([P, NKT, HC, VW], bf16)  # ~16.6KB/part
            wo_sb = persist.tile([P, NFC, D], f32r)       # 16KB/part
            bq_sb = persist.tile([P, NFC], f32)
            bk_sb = persist.tile([P, NFC], f32)
            bvb = persist.tile([P, DC], f32)              # bias_v broadcast

            nc.sync.dma_start(out=bq_sb, in_=bq.rearrange("(c p) -> p c", p=P))
            nc.sync.dma_start(out=bk_sb, in_=bk.rearrange("(c p) -> p c", p=P))
            bv_ap = bv.ap()
            bvb_src = bass.AP(
                tensor=bv_ap.tensor, offset=bv_ap.offset, ap=[[0, P], *bv_ap.ap]
            )
            nc.sync.dma_start(out=bvb, in_=bvb_src)
            # ones column (Z trick) + f32r ones row for the 1/Z broadcast MM
            ones_st = persist.tile([P, P], f32)
            nc.vector.memset(ones_st, 1.0)
            nc.vector.tensor_copy(
                out=v_aug[:, :, :, DK],
                in_=ones_st.rearrange("p (k h) -> p k h", k=NKT),
            )
            ones_r = persist.tile([P, DK], f32r)
            nc.vector.tensor_copy(out=ones_r, in_=ones_st[:, 0:DK])

            # ---- emission helpers (PE program order == emission order) ----
            def load_w(w_dram, name, tag="w", bufs=None, split=False):
                w_sb = wpool.tile([P, NDCH, DC], bf16, tag=tag, name=name, bufs=bufs)
                wr = w_dram.rearrange("(c p) f -> p c f", p=P)
                if split:
                    h_ = NDCH // 2
                    nc.sync.dma_start(out=w_sb[:, 0:h_, :], in_=wr[:, 0:h_, :])
                    nc.sync.dma_start(out=w_sb[:, h_:, :], in_=wr[:, h_:, :])
                else:
                    nc.sync.dma_start(out=w_sb, in_=wr)
                return w_sb

            def load_x(xT_dram, g, name, tag="x", bufs=None, split=False):
                x_sb = xpool.tile([P, NDCH, QG], bf16, tag=tag, name=name, bufs=bufs)
                xr = xT_dram.rearrange("(c p) t -> p c t", p=P)[:, :, ts(g, QG)]
                if split:
                    h_ = NDCH // 2
                    nc.sync.dma_start(out=x_sb[:, 0:h_, :], in_=xr[:, 0:h_, :])
                    nc.sync.dma_start(out=x_sb[:, h_:, :], in_=xr[:, h_:, :])
                else:
                    nc.sync.dma_start(out=x_sb, in_=xr)
                return x_sb

            def kproj_chain(w_sb, x_sb, g, fc):
                ps = pp.tile([P, QG], f32, tag="pp", name=f"pk_{g}_{fc}")
                for dch in range(NDCH):
                    nc.tensor.matmul(
                        ps, w_sb[:, dch, ts(fc, P)], x_sb[:, dch, :],
                        start=(dch == 0), stop=(dch == NDCH - 1),
                    )
                nc.vector.tensor_scalar(
                    out=kT[:, fc, ts(g, QG)], in0=ps,
                    scalar1=bk_sb[:, fc : fc + 1], scalar2=None, op0=ALU.add,
                )

            def qproj_chain(w_sb, x_sb, qT, g, fc):
                ps = pp.tile([P, QG], f32, tag="pp", name=f"pq_{g}_{fc}")
                for dch in range(NDCH):
                    nc.tensor.matmul(
                        ps, w_sb[:, dch, ts(fc, P)], x_sb[:, dch, :],
                        start=(dch == 0), stop=(dch == NDCH - 1),
                    )
                nc.vector.tensor_scalar(
                    out=qT[:, fc, :], in0=ps,
                    scalar1=bq_sb[:, fc : fc + 1], scalar2=None, op0=ALU.add,
                )

            def vproj_tile(w_sb, x_sb, kt):
                tt = kt % (QG // P)
                ps = pp.tile([P, DC], f32, tag="pp", name=f"pv_{kt}")
                for dch in range(NDCH):
                    nc.tensor.matmul(
                        ps, x_sb[:, dch, ts(tt, P)], w_sb[:, dch, :],
                        start=(dch == 0), stop=(dch == NDCH - 1),
                    )
                nc.vector.tensor_add(
                    out=v_aug[:, kt, :, 0:DK],
                    in0=ps.rearrange("p (h d) -> p h d", h=HC),
                    in1=bvb.rearrange("p (h d) -> p h d", h=HC),
                )

            def outproj_chain(oT, g, tt, eg, pool=None):
                pool = pool or pp
                ps = pool.tile(
                    [P, DC], f32, tag="pp" if pool is pp else "av",
                    name=f"po_{g}_{tt}_{eg}",
                )
                for fc in range(NFC):
                    nc.tensor.matmul(
                        ps, oT[:, fc, ts(tt, P)], wo_sb[:, fc, ts(eg, DC)],
                        start=(fc == 0), stop=(fc == NFC - 1),
                    )
                o_sb = osb_pool.tile([P, DC], f32, tag="osb", name=f"ob_{g}_{tt}_{eg}")
                nc.vector.tensor_copy(out=o_sb, in_=ps)
                nc.sync.dma_start(
                    out=out[ds(g * QG + tt * P, P), ts(eg, DC)], in_=o_sb
                )

            def attention_pair(
                g, hp, qT, oT, splice=None, pre_attv=None, splice_post=None
            ):
                """scores+exp for head pair (2hp, 2hp+1), then att@V + norm.

                splice(kt2): extra PE work emitted before kt2's score MMs
                (used to interleave the k-projection into pair 0 just-in-
                time: block b must be written before scores at kt2=2b).
                pre_attv(): emitted between the score loop and att@V
                (used for the v projection, which att@V needs in full).
                """
                ha, hb = 2 * hp, 2 * hp + 1
                ests = {ha: [], hb: []}
                for kt2 in range(NKT // 2):
                    if splice is not None:
                        splice(kt2)
                    sts = {
                        h: st_pool.tile(
                            [P, 2, QG], f32, tag="st", name=f"st_{g}_{h}_{kt2}"
                        )
                        for h in (ha, hb)
                    }
                    for kk in range(2):
                        kt = 2 * kt2 + kk
                        for h in (ha, hb):
                            r0 = (h % 2) * DK
                            nc.tensor.matmul(
                                sts[h][:, kk, :],
                                kT[r0 : r0 + DK, hp, ts(kt, P)],
                                qT[r0 : r0 + DK, hp, :],
                                start=True, stop=True, tile_position=(r0, 0),
                            )
                    for h in (ha, hb):
                        e = epool.tile(
                            [P, 2, QG], bf16, tag="est", name=f"est_{g}_{h}_{kt2}"
                        )
                        ests[h].append(e)
                        nc.scalar.activation(
                            out=e, in_=sts[h], func=AF.Exp, scale=INV_SCALE
                        )
                    if splice_post is not None:
                        splice_post(kt2)
                if pre_attv is not None:
                    pre_attv()
                for h in (ha, hb):
                    av = avp.tile([P, QG], f32, tag="av", name=f"av_{g}_{h}")
                    for kt in range(NKT):
                        nc.tensor.matmul(
                            av[0:VW, :],
                            v_aug[:, kt, h, :],
                            ests[h][kt // 2][:, kt % 2, :],
                            start=(kt == 0), stop=(kt == NKT - 1),
                        )
                    # copy [out; Z] to SBUF right away so the av PSUM bank
                    # frees for the next pair's att@V; the normalize multiply
                    # then reads the 1/Z broadcast directly from PSUM
                    avs = spool.tile([P, QG], f32, tag="avs", name=f"avs_{g}_{h}")
                    nc.vector.tensor_copy(out=avs[0:VW, :], in_=av[0:VW, :])
                    rz = spool.tile([P, QG], f32r, tag="rz", name=f"rz_{g}_{h}")
                    with nc.allow_low_precision("tf32 softmax denom"):
                        nc.vector.reciprocal(
                            out=rz[DK : DK + 1, :], in_=avs[DK : DK + 1, :]
                        )
                    rzb_ps = pp.tile([P, QG], f32, tag="pp", name=f"rzp_{g}_{h}")
                    nc.tensor.matmul(
                        rzb_ps[0:DK, :],
                        ones_r[DK : DK + 1, 0:DK],
                        rz[DK : DK + 1, :],
                        start=True, stop=True, tile_position=(DK, 0),
                    )
                    if h % 2 == 0:
                        nc.vector.tensor_mul(
                            out=oT[0:DK, hp, :],
                            in0=avs[0:DK, :],
                            in1=rzb_ps[0:DK, :],
                        )
                    else:
                        tmp = spool.tile([P, QG], f32r, tag="rz", name=f"tmp_{g}_{h}")
                        nc.vector.tensor_mul(
                            out=tmp[0:DK, :], in0=avs[0:DK, :], in1=rzb_ps[0:DK, :]
                        )
                        nc.sync.dma_start(out=oT[DK:P, hp, :], in_=tmp[0:DK, :])

            # ---- prelude: k projection (scores need kT in full) ----
            # critical-path DMAs first: wk/xk0 feed the first chains, wq/xq0
            # unblock the first q-projection right after kproj ends
            wk_sb = load_w(wk, "w_k", split=True)
            xk_sbs = [load_x(xkT, 0, "x_k_0", split=True)]
            wq_sb = load_w(wq, "w_q", tag="wq", bufs=1)
            xq_first = load_x(xqT, 0, "x_q_0", tag="xq", bufs=1)
            for g in range(NQG):
                if g + 1 < NQG:
                    xk_sbs.append(load_x(xkT, g + 1, f"x_k_{g + 1}"))
                for fc in range(NFC):
                    kproj_chain(wk_sb, xk_sbs[g], g, fc)

            # v weight next; wo late (first needed by outproj of group 0)
            wv_sb = load_w(wv, "w_v")
            nc.sync.dma_start(out=wo_sb, in_=wo.rearrange("(c p) e -> p c e", p=P))

            # v-projection splice for group 0 pair 0 (xv0 prefetched)
            xv_tiles = {0: load_x(xvT, 0, "x_v_0")}

            def post0(kt2):
                # v projection emitted AFTER each kt2's scores+exps: scores
                # stay one chain ahead of ACT, and v completes by loop end
                for kk in range(2):
                    kt = 2 * kt2 + kk
                    gg = kt // (QG // P)
                    if gg not in xv_tiles:
                        xv_tiles[gg] = load_x(xvT, gg, f"x_v_{gg}")
                    vproj_tile(wv_sb, xv_tiles[gg], kt)

            def splice0(kt2):
                if kt2 == 2:
                    # qproj c1 (first needed by pair 1), deferred off the
                    # ScalarE start path
                    qproj_chain(wq_sb, qst[0][0], qst[0][1], 0, 1)
                elif kt2 == 4:
                    # pair 0 of group 0 skips the generic qproj splice slot,
                    # so emit group 0's chunk-2 chain here (pair 2 needs it)
                    qproj_chain(wq_sb, qst[0][0], qst[0][1], 0, 2)

            prev = None  # (g, oT) pending output projection
            # one-group lookahead: (xq, qT) for group g+1 are created and
            # their first two qproj chains spliced into group g's pairs 2/3,
            # so group boundaries leave no PE work ahead of the next scores
            qst = {0: (xq_first, qpool.tile([P, NFC, QG], bf16, tag="qT", name="qT_0"))}
            qproj_chain(wq_sb, qst[0][0], qst[0][1], 0, 0)
            for g in range(NQG):
                xq_sb, qT = qst[g]
                oT = opool.tile([P, NFC, QG], f32r, tag="oT", name=f"oT_{g}")
                for hp in range(NHP):
                    # qproj / previous-group outproj chains are spliced into
                    # this pair's score loop (PE has slack there: ~1.4us of
                    # work per kt2 vs ACT's 2.3us exp cadence), so ScalarE
                    # never waits at pair boundaries.
                    def mksplice(g=g, hp=hp, qT=qT, xq_sb=xq_sb, prev=prev):
                        def splice(kt2):
                            if g == 0 and hp == 0:
                                splice0(kt2)
                                return
                            if kt2 == 1:
                                if hp + 2 < NFC:
                                    qproj_chain(wq_sb, xq_sb, qT, g, hp + 2)
                                elif g + 1 < NQG:
                                    if g + 1 not in qst:
                                        qst[g + 1] = (
                                            load_x(
                                                xqT, g + 1, f"x_q_{g + 1}",
                                                tag="xq", bufs=1,
                                            ),
                                            qpool.tile(
                                                [P, NFC, QG], bf16, tag="qT",
                                                name=f"qT_{g + 1}",
                                            ),
                                        )
                                    nx, nq = qst[g + 1]
                                    qproj_chain(wq_sb, nx, nq, g + 1, hp - 2)
                            if prev is not None:
                                pg, poT = prev
                                if kt2 == 3:
                                    outproj_chain(poT, pg, hp, 0)
                                elif kt2 == 5:
                                    outproj_chain(poT, pg, hp, 1)
                        return splice

                    attention_pair(
                        g, hp, qT, oT, splice=mksplice(),
                        splice_post=post0 if (g == 0 and hp == 0) else None,
                    )
                prev = (g, oT)
            # tail: output projection for the last group — alternate the
            # two PSUM pools (score pipeline is done, its banks are idle)
            # for 4-deep chain pipelining
            pg, poT = prev
            for i, (tt, eg) in enumerate(
                (tt, eg) for tt in range(QG // P) for eg in range(2)
            ):
                outproj_chain(poT, pg, tt, eg, pool=(pp if i % 2 == 0 else avp))

    nc.compile()
    return nc


def _get_nc(debug=False):
    if "nc" not in _CACHE:
        _CACHE["nc"] = _build()
    return _CACHE["nc"]


def _tf32(a):
    """Round fp32 to the TF32 grid (10-bit mantissa, round-to-nearest-even)."""
    u = np.ascontiguousarray(a, dtype=np.float32).view(np.uint32)
    u = (u + np.uint32(0xFFF) + ((u >> np.uint32(13)) & np.uint32(1))) & np.uint32(
        0xFFFFE000
    )
    return u.view(np.float32)


def _bf16(a):
    import ml_dtypes

    return np.ascontiguousarray(a, dtype=np.float32).astype(ml_dtypes.bfloat16)


def _make_in_maps(inputs):
    q = np.asarray(inputs["query"], dtype=np.float32)
    k = np.asarray(inputs["key"], dtype=np.float32)
    v = np.asarray(inputs["value"], dtype=np.float32)
    wq = np.asarray(inputs["wq"], dtype=np.float32)
    wk = np.asarray(inputs["wk"], dtype=np.float32)
    wv = np.asarray(inputs["wv"], dtype=np.float32)
    wo = np.asarray(inputs["wo"], dtype=np.float32)
    bq = np.asarray(inputs["bq"], dtype=np.float32)
    bk = np.asarray(inputs["bk"], dtype=np.float32)
    bv = np.asarray(inputs["bv"], dtype=np.float32)

    xT = [(_bf16(q[b].T), _bf16(k[b].T), _bf16(v[b].T)) for b in range(B)]
    in_maps = []
    for c in range(NCORES):
        b, g = divmod(c, 2)
        sl = slice(g * DC, (g + 1) * DC)
        in_maps.append(
            {
                "xqT": xT[b][0],
                "xkT": xT[b][1],
                "xvT": xT[b][2],
                "wq": _bf16(wq[:, sl]),
                "wk": _bf16(wk[:, sl]),
                "wv": _bf16(wv[:, sl]),
                "wo": _tf32(wo[sl, :]),
                "bq": np.ascontiguousarray(bq[sl]),
                "bk": np.ascontiguousarray(bk[sl]),
                "bv": np.ascontiguousarray(bv[sl]),
            }
        )
    return in_maps


def run(inputs, **kwargs):
    """Run the kernel; returns (full_output, BassKernelResults)."""
    from concourse.bass_utils import run_bass_kernel_spmd

    kwargs.pop("debug", None)
    nc = _get_nc()
    in_maps = _make_in_maps(inputs)
    res = run_bass_kernel_spmd(nc, in_maps, core_ids=list(range(NCORES)), **kwargs)
    bo = np.asarray(inputs["bo"], dtype=np.float32)
    final = np.empty((B, S, D), np.float32)
    for b in range(B):
        final[b] = res.results[2 * b]["out"] + res.results[2 * b + 1]["out"] + bo
    return final, res


def kernel(**inputs):
    return run(inputs)[0]



# revision 9
# speedup vs baseline: 1.0788x; 1.0788x over previous
"""Multi-head attention (B=4, S=2048, D=1024, H=16) on 8 TRN2 NeuronCores.

Sharding (Megatron-style, per spec hint): data-parallel over batch (4) x
tensor-parallel over heads (2 groups of 8). Core c handles batch c//2,
head-group c%2. QKV projections column-sharded, output projection
row-sharded; the two partial outputs per batch are summed on the host
together with the output bias.

Per-core kernel (one NeuronCore, 8 heads, 2048 tokens), v2:
  - Scores transposed ST[k, q] with softmax-exp (no max subtraction) as one
    ACT pass per [128, 2, 512] score tile, bf16 out.
  - att@V uses the probabilities as the STATIONARY operand ([128k, 128q]
    slices) and v tiles [128k, 64] as moving, so the output [128q, 64]
    fills all 128 PSUM partitions: half the PE cost of the v-stationary
    form. A head-pair's whole output (4 q-tiles x 2 heads x 64) packs into
    exactly one PSUM bank with a single accumulation start/stop.
  - The softmax denominator Z accumulates via 1-column matmuls against the
    v_aug ones column into a separate z bank.
  - Normalration is one DVE pass per pair (stride-0 broadcast of 1/Z);
    the normalized [q, feature] tiles are transposed back to feature-major
    by the DMA xbar (dma_start_transpose), not the PE.
  - att@V chains are spliced into the NEXT pair's score loop (PE slack per
    kt2 slot), so ScalarE streams exps with few gaps; k/v/q projections and
    the previous group's output projection are spliced the same way.
"""

import sys

if "/opt/trn_rl_repo" not in sys.path:
    sys.path.insert(0, "/opt/trn_rl_repo")

import numpy as np

B, S, D = 4, 2048, 1024
H, DK = 16, 64
NCORES = 8
HC = H // 2            # heads per core
DC = HC * DK           # 512 local features per core
INV_SCALE = 1.0 / 8.0  # 1/sqrt(DK)
P = 128
NDCH = D // P          # 8 contraction chunks for projections
NFC = DC // P          # 4 local feature chunks
NKT = S // P           # 16 key tiles
NQG = 4                # query groups
QG = S // NQG          # 512 queries per group
NQT = QG // P          # 4 query tiles per group
VW = DK + 1            # 65: v columns + ones column
NHP = HC // 2          # head pairs

_CACHE = {}


def _build():
    import concourse.bass as bass
    import concourse.bacc as bacc
    import concourse.tile as tile
    import concourse.mybir as mybir
    from concourse.bass import ts, ds

    f32 = mybir.dt.float32
    f32r = mybir.dt.float32r
    bf16 = mybir.dt.bfloat16
    AF = mybir.ActivationFunctionType
    ALU = mybir.AluOpType

    nc = bacc.Bacc("TRN2", target_bir_lowering=False, num_devices=NCORES)

    xqT = nc.dram_tensor("xqT", [D, S], bf16, kind="ExternalInput")
    xkT = nc.dram_tensor("xkT", [D, S], bf16, kind="ExternalInput")
    xvT = nc.dram_tensor("xvT", [D, S], bf16, kind="ExternalInput")
    wq = nc.dram_tensor("wq", [D, DC], bf16, kind="ExternalInput")
    wk = nc.dram_tensor("wk", [D, DC], bf16, kind="ExternalInput")
    wv = nc.dram_tensor("wv", [D, DC], bf16, kind="ExternalInput")
    wo = nc.dram_tensor("wo", [DC, D], bf16, kind="ExternalInput")
    bq = nc.dram_tensor("bq", [DC], f32, kind="ExternalInput")
    bk = nc.dram_tensor("bk", [DC], f32, kind="ExternalInput")
    bv = nc.dram_tensor("bv", [DC], f32, kind="ExternalInput")
    out = nc.dram_tensor("out", [S, D], bf16, kind="ExternalOutput")

    with tile.TileContext(nc) as tc:
        with (
            tc.tile_pool(name="persist", bufs=1) as persist,
            tc.tile_pool(name="wts", bufs=2) as wpool,
            tc.tile_pool(name="xin", bufs=4) as xpool,
            tc.tile_pool(name="qt", bufs=2) as qpool,
            tc.tile_pool(name="expst", bufs=18) as epool,
            tc.tile_pool(name="osb", bufs=2) as ospool,
            tc.tile_pool(name="att", bufs=1) as atpool,
            tc.tile_pool(name="small", bufs=2) as spool,
            tc.tile_pool(name="oc", bufs=2) as ocpool,
            tc.tile_pool(name="pp", bufs=2, space="PSUM") as pp,
            tc.tile_pool(name="st", bufs=2, space="PSUM") as st_pool,
            tc.tile_pool(name="av", bufs=1, space="PSUM") as avp,
            tc.tile_pool(name="zp", bufs=1, space="PSUM") as zpool,
        ):
            # ---- persistent SBUF tensors ----
            kT = persist.tile([P, NFC, S], bf16)          # 16KB/part
            v_aug = persist.tile([P, NKT, HC, VW], bf16)  # ~16.6KB/part
            wo_sb = persist.tile([P, NFC, D], bf16)       # 8KB/part
            bq_sb = persist.tile([P, NFC], f32)
            bk_sb = persist.tile([P, NFC], f32)
            bvb = persist.tile([P, DC], f32)              # bias_v broadcast

            nc.sync.dma_start(out=bq_sb, in_=bq.rearrange("(c p) -> p c", p=P))
            nc.sync.dma_start(out=bk_sb, in_=bk.rearrange("(c p) -> p c", p=P))
            bv_ap = bv.ap()
            bvb_src = bass.AP(
                tensor=bv_ap.tensor, offset=bv_ap.offset, ap=[[0, P], *bv_ap.ap]
            )
            nc.sync.dma_start(out=bvb, in_=bvb_src)
            # ones column of v_aug (softmax denominator trick)
            ones_st = persist.tile([P, P], f32)
            nc.vector.memset(ones_st, 1.0)
            nc.vector.tensor_copy(
                out=v_aug[:, :, :, DK],
                in_=ones_st.rearrange("p (k h) -> p k h", k=NKT),
            )

            # ---- emission helpers (PE program order == emission order) ----
            def load_w(w_dram, name, tag="w", bufs=None, fc_split=False,
                       defer=False):
                w_sb = wpool.tile([P, NDCH, DC], bf16, tag=tag, name=name, bufs=bufs)
                wr = w_dram.rearrange("(c p) f -> p c f", p=P)
                if fc_split:
                    # first half of the feature chunks now; rest via thunk
                    nc.sync.dma_start(out=w_sb[:, :, 0:DC // 2], in_=wr[:, :, 0:DC // 2])
                    rest = lambda: nc.sync.dma_start(
                        out=w_sb[:, :, DC // 2:], in_=wr[:, :, DC // 2:])
                    if defer:
                        return w_sb, rest
                    rest()
                else:
                    nc.sync.dma_start(out=w_sb, in_=wr)
                return w_sb

            def load_x(xT_dram, g, name, tag="x", bufs=None, split=False):
                x_sb = xpool.tile([P, NDCH, QG], bf16, tag=tag, name=name, bufs=bufs)
                xr = xT_dram.rearrange("(c p) t -> p c t", p=P)[:, :, ts(g, QG)]
                if split:
                    h_ = NDCH // 2
                    nc.sync.dma_start(out=x_sb[:, 0:h_, :], in_=xr[:, 0:h_, :])
                    nc.sync.dma_start(out=x_sb[:, h_:, :], in_=xr[:, h_:, :])
                else:
                    nc.sync.dma_start(out=x_sb, in_=xr)
                return x_sb

            def kproj_chain(w_sb, x_sb, g, fc):
                ps = pp.tile([P, QG], f32, tag="pp", name=f"pk_{g}_{fc}")
                for dch in range(NDCH):
                    nc.tensor.matmul(
                        ps, w_sb[:, dch, ts(fc, P)], x_sb[:, dch, :],
                        start=(dch == 0), stop=(dch == NDCH - 1),
                    )
                nc.vector.tensor_scalar(
                    out=kT[:, fc, ts(g, QG)], in0=ps,
                    scalar1=bk_sb[:, fc : fc + 1], scalar2=None, op0=ALU.add,
                )

            def qproj_chain(w_sb, x_sb, qT, g, fc):
                ps = pp.tile([P, QG], f32, tag="pp", name=f"pq_{g}_{fc}")
                for dch in range(NDCH):
                    nc.tensor.matmul(
                        ps, w_sb[:, dch, ts(fc, P)], x_sb[:, dch, :],
                        start=(dch == 0), stop=(dch == NDCH - 1),
                    )
                nc.vector.tensor_scalar(
                    out=qT[:, fc, :], in0=ps,
                    scalar1=bq_sb[:, fc : fc + 1], scalar2=None, op0=ALU.add,
                )

            def vproj_tile(w_sb, x_sb, kt):
                tt = kt % NQT
                ps = pp.tile([P, DC], f32, tag="pp", name=f"pv_{kt}")
                for dch in range(NDCH):
                    nc.tensor.matmul(
                        ps, x_sb[:, dch, ts(tt, P)], w_sb[:, dch, :],
                        start=(dch == 0), stop=(dch == NDCH - 1),
                    )
                nc.vector.tensor_add(
                    out=v_aug[:, kt, :, 0:DK],
                    in0=ps.rearrange("p (h d) -> p h d", h=HC),
                    in1=bvb.rearrange("p (h d) -> p h d", h=HC),
                )

            def outproj_chain(attnT, g, tt, eg, pool=None):
                pool = pool or pp
                ps = pool.tile(
                    [P, DC], f32, tag="pp" if pool is pp else "av",
                    name=f"po_{g}_{tt}_{eg}",
                )
                for fc in range(NFC):
                    nc.tensor.matmul(
                        ps, attnT[:, fc, ts(tt, P)], wo_sb[:, fc, ts(eg, DC)],
                        start=(fc == 0), stop=(fc == NFC - 1),
                    )
                o_sb = ocpool.tile([P, DC], bf16, tag="osb", name=f"ob_{g}_{tt}_{eg}")
                nc.vector.tensor_copy(out=o_sb, in_=ps)
                nc.sync.dma_start(
                    out=out[ds(g * QG + tt * P, P), ts(eg, DC)], in_=o_sb
                )

            # ---- pair state: est tiles + av/z banks, consumed one pair later
            class PairState:
                def __init__(self, g, hp):
                    self.g, self.hp = g, hp
                    self.ests = {}   # h -> list of 8 est tiles [P, 2, QG]
                    self.av = None   # [P, NQT, 2, DK] f32 psum (1 bank)
                    self.zt = None   # [P, QG] f32 psum (1 bank; cols 0:8 used)

            def attv_slice(ps_, s):
                """att@V + Z matmuls consuming est[s] (key tiles 2s, 2s+1)."""
                g, hp = ps_.g, ps_.hp
                if s == 0:
                    ps_.av = avp.tile(
                        [P, NQT, 2, DK], f32, tag="av", name=f"av_{g}_{hp}"
                    )
                    ps_.zt = zpool.tile([P, QG], f32, tag="z", name=f"z_{g}_{hp}")
                last = NKT // 2 - 1
                for kk in range(2):
                    kt = 2 * s + kk
                    for qt in range(NQT):
                        for hh in range(2):
                            h = 2 * hp + hh
                            est = ps_.ests[h][s]
                            stat = est[:, kk, ts(qt, P)]
                            first = s == 0 and kk == 0 and qt == 0 and hh == 0
                            lastm = s == last and kk == 1 and qt == NQT - 1 and hh == 1
                            nc.tensor.matmul(
                                ps_.av[:, qt, hh, :], stat,
                                v_aug[:, kt, h, 0:DK],
                                start=first, stop=lastm,
                            )
                            c = qt * 2 + hh
                            nc.tensor.matmul(
                                ps_.zt[:, c : c + 1], stat,
                                v_aug[:, kt, h, DK:VW],
                                start=first, stop=lastm,
                            )

            def finish_pair(ps_, o_sb_tiles):
                """reciprocal + normalize for a finished pair."""
                g, hp = ps_.g, ps_.hp
                rz = spool.tile([P, NQT, 2], f32r, tag="rz", name=f"rz_{g}_{hp}")
                with nc.allow_low_precision("softmax denom reciprocal"):
                    nc.vector.reciprocal(
                        out=rz,
                        in_=ps_.zt[:, 0 : 2 * NQT].rearrange(
                            "p (q h) -> p q h", q=NQT
                        ),
                    )
                o_sb = o_sb_tiles[g]
                nc.vector.tensor_tensor(
                    out=o_sb[:, :, 2 * hp : 2 * hp + 2, :],
                    in0=ps_.av,
                    in1=rz.unsqueeze(-1).broadcast_to([P, NQT, 2, DK]),
                    op=ALU.mult,
                )

            def transposes(g, o_sb_tiles, attnT):
                o_sb = o_sb_tiles[g]
                for qt in range(NQT):
                    for fc in range(NFC):
                        nc.sync.dma_start_transpose(
                            out=attnT[:, fc, ts(qt, P)],
                            in_=o_sb[:, qt, 2 * fc : 2 * fc + 2, :],
                        )

            # =========== prelude ===========
            # DMA order tuned so the first-score chain (wk fc01, xk0, wq
            # fc01, xq0) clears in ~10us and fill-phase consumers (xv0, wv,
            # xk1-3) arrive before their spliced chains need them.
            wk_sb, wk_rest = load_w(wk, "w_k", fc_split=True, defer=True)
            xk_sbs = [load_x(xkT, 0, "x_k_0", tag="xk", bufs=4, split=True)]
            wq_sb, wq_rest = load_w(wq, "w_q", tag="wq", bufs=1, fc_split=True,
                                    defer=True)
            xq_tiles = {0: load_x(xqT, 0, "x_q_0", tag="xq", bufs=2)}
            kproj_chain(wk_sb, xk_sbs[0], 0, 0)

            qst = {0: qpool.tile([P, NFC, QG], bf16, tag="qT", name="qT_0")}
            qproj_chain(wq_sb, xq_tiles[0], qst[0], 0, 0)

            xk_sbs.append(load_x(xkT, 1, "x_k_1", tag="xk", bufs=4))
            wv_sb = load_w(wv, "w_v")
            xv_tiles = {0: load_x(xvT, 0, "x_v_0", tag="xv", bufs=2)}
            xk_sbs.append(load_x(xkT, 2, "x_k_2", tag="xk", bufs=4))
            xk_sbs.append(load_x(xkT, 3, "x_k_3", tag="xk", bufs=4))
            wk_rest()
            wq_rest()
            nc.sync.dma_start(out=wo_sb, in_=wo.rearrange("(c p) e -> p c e", p=P))

            # =========== splice schedule ===========
            # pair index p = 4*g + hp runs score loop slots 0..7; sched[p][s]
            # is a list of thunks emitted before slot s's score matmuls.
            sched = {p: {s: [] for s in range(8)} for p in range(16)}

            def at(p, s, fn):
                sched[p][s].append(fn)

            # kproj: fc=0 for kg>=1 early in pair 0; fc=f in pair f-1... but
            # pair (0,hp) reads kT chunk hp for all kt: chunk fc must be fully
            # projected (all 4 kg) before pair (0,fc) starts.
            for kg, s_ in [(1, 0), (2, 1), (3, 3)]:
                at(0, s_, lambda kg=kg: kproj_chain(wk_sb, xk_sbs[kg], kg, 0))
            for fc in range(1, 4):
                for kg in range(4):
                    at(fc - 1, 2 * kg + 1, lambda kg=kg, fc=fc: kproj_chain(
                        wk_sb, xk_sbs[kg], kg, fc))
            # vproj: 10 tiles in pair 0 (extra on later slots), 6 in pair 1;
            # v_aug[kt] needed by attV(0,0) slice s=kt//2 at pair 1 slot s.
            # xv loads run >=2 slots ahead of their first vproj consumer.
            for vg, (p_, s_) in {1: (0, 1), 2: (0, 5), 3: (0, 7)}.items():
                at(p_, s_, lambda vg=vg: xv_tiles.__setitem__(
                    vg, load_x(xvT, vg, f"x_v_{vg}", tag="xv", bufs=2)))
            vq = [(0, 0, 1), (0, 1, 1), (0, 2, 1), (0, 3, 1), (0, 4, 2),
                  (0, 5, 2), (0, 6, 2), (0, 7, 2), (1, 0, 2), (1, 1, 2),
                  (1, 2, 2)]
            kt_next = 0
            for p_, s_, n_ in vq:
                for _ in range(n_):
                    if kt_next >= NKT:
                        break
                    kt = kt_next
                    kt_next += 1
                    at(p_, s_, lambda kt=kt: vproj_tile(
                        wv_sb, xv_tiles[kt // NQT], kt))
            # qproj for pair p+1 at pair p slot 5 (+ xq loads 2 pairs early)
            for p in range(15):
                g1, fc1 = divmod(p + 1, 4)
                if fc1 == 0 and g1 > 0:
                    at(p - 2 if p >= 2 else 0, 1, lambda g1=g1: xq_tiles.__setitem__(
                        g1, load_x(xqT, g1, f"x_q_{g1}", tag="xq", bufs=2)))
                    at(p, 5, lambda g1=g1: (
                        qst.__setitem__(g1, qpool.tile(
                            [P, NFC, QG], bf16, tag="qT", name=f"qT_{g1}")),
                        qproj_chain(wq_sb, xq_tiles[g1], qst[g1], g1, 0))[-1])
                else:
                    at(p, 5, lambda g1=g1, fc1=fc1: qproj_chain(
                        wq_sb, xq_tiles[g1], qst[g1], g1, fc1))
            # outproj(g) chains spliced into pairs of group g+1
            op_slots = [(1, 4), (1, 6), (2, 2), (2, 4), (2, 6), (3, 2),
                        (3, 4), (3, 6)]
            attnT_holder = {}
            for g in range(3):
                for i, (hp_, s_) in enumerate(op_slots):
                    tt, eg = divmod(i, 2)
                    at(4 * (g + 1) + hp_, s_, lambda g=g, tt=tt, eg=eg: outproj_chain(
                        attnT_holder[g], g, tt, eg))

            # =========== main loop ===========
            o_sb_tiles = {}
            prev_pair = None   # PairState consumed by current pair's splices
            done_pair = None   # PairState whose attV completed last pair
            # (its finish_pair runs at the START of this pair so the DVE
            # queue never parks on unmet deps — DVE is in-order)

            for p in range(16):
                g, hp = divmod(p, 4)
                if g not in o_sb_tiles:
                    o_sb_tiles[g] = ospool.tile(
                        [P, NQT, HC, DK], bf16, tag="osb2", name=f"o_{g}"
                    )
                cur = PairState(g, hp)
                qT = qst[g]
                for kt2 in range(NKT // 2):
                    if kt2 == 0 and done_pair is not None:
                        finish_pair(done_pair, o_sb_tiles)
                        if done_pair.hp == NHP - 1:
                            gg = done_pair.g
                            attnT_holder[gg] = atpool.tile(
                                [P, NFC, QG], bf16, tag="attnT", name=f"aT_{gg}"
                            )
                            transposes(gg, o_sb_tiles, attnT_holder[gg])
                        done_pair = None
                    # splices: attV of previous pair, then scheduled items
                    if prev_pair is not None:
                        attv_slice(prev_pair, kt2)
                    for fn in sched[p][kt2]:
                        fn()
                    # score matmuls for this slot
                    sts = {}
                    for hh in range(2):
                        h = 2 * hp + hh
                        sts[h] = st_pool.tile(
                            [P, 2, QG], f32, tag="st", name=f"st_{g}_{h}_{kt2}"
                        )
                    for kk in range(2):
                        kt = 2 * kt2 + kk
                        for hh in range(2):
                            h = 2 * hp + hh
                            r0 = hh * DK
                            nc.tensor.matmul(
                                sts[h][:, kk, :],
                                kT[r0 : r0 + DK, hp, ts(kt, P)],
                                qT[r0 : r0 + DK, hp, :],
                                start=True, stop=True, tile_position=(r0, 0),
                            )
                    for hh in range(2):
                        h = 2 * hp + hh
                        e = epool.tile(
                            [P, 2, QG], bf16, tag="est", name=f"est_{g}_{h}_{kt2}"
                        )
                        cur.ests.setdefault(h, []).append(e)
                        nc.scalar.activation(
                            out=e, in_=sts[h], func=AF.Exp, scale=INV_SCALE
                        )
                # previous pair's attV is complete; finish it at the start
                # of the next pair (deps met there, no DVE queue parking)
                done_pair = prev_pair
                prev_pair = cur

            # =========== tail: last pair's attV + outproj of group 3 ========
            finish_pair(done_pair, o_sb_tiles)
            for s in range(NKT // 2):
                attv_slice(prev_pair, s)
            finish_pair(prev_pair, o_sb_tiles)
            attnT_holder[3] = atpool.tile(
                [P, NFC, QG], bf16, tag="attnT", name="aT_3"
            )
            transposes(3, o_sb_tiles, attnT_holder[3])
            for i, (tt, eg) in enumerate(
                (tt, eg) for tt in range(NQT) for eg in range(2)
            ):
                outproj_chain(
                    attnT_holder[3], 3, tt, eg,
                    pool=(pp if i % 2 == 0 else avp),
                )

    nc.compile()
    return nc


def _get_nc(debug=False):
    if "nc" not in _CACHE:
        _CACHE["nc"] = _build()
    return _CACHE["nc"]


def _tf32(a):
    """Round fp32 to the TF32 grid (10-bit mantissa, round-to-nearest-even)."""
    u = np.ascontiguousarray(a, dtype=np.float32).view(np.uint32)
    u = (u + np.uint32(0xFFF) + ((u >> np.uint32(13)) & np.uint32(1))) & np.uint32(
        0xFFFFE000
    )
    return u.view(np.float32)


def _bf16(a):
    import ml_dtypes

    return np.ascontiguousarray(a, dtype=np.float32).astype(ml_dtypes.bfloat16)


def _make_in_maps(inputs):
    q = np.asarray(inputs["query"], dtype=np.float32)
    k = np.asarray(inputs["key"], dtype=np.float32)
    v = np.asarray(inputs["value"], dtype=np.float32)
    wq = np.asarray(inputs["wq"], dtype=np.float32)
    wk = np.asarray(inputs["wk"], dtype=np.float32)
    wv = np.asarray(inputs["wv"], dtype=np.float32)
    wo = np.asarray(inputs["wo"], dtype=np.float32)
    bq = np.asarray(inputs["bq"], dtype=np.float32)
    bk = np.asarray(inputs["bk"], dtype=np.float32)
    bv = np.asarray(inputs["bv"], dtype=np.float32)

    xT = [(_bf16(q[b].T), _bf16(k[b].T), _bf16(v[b].T)) for b in range(B)]
    in_maps = []
    for c in range(NCORES):
        b, g = divmod(c, 2)
        sl = slice(g * DC, (g + 1) * DC)
        in_maps.append(
            {
                "xqT": xT[b][0],
                "xkT": xT[b][1],
                "xvT": xT[b][2],
                "wq": _bf16(wq[:, sl]),
                "wk": _bf16(wk[:, sl]),
                "wv": _bf16(wv[:, sl]),
                "wo": _bf16(wo[sl, :]),
                "bq": np.ascontiguousarray(bq[sl]),
                "bk": np.ascontiguousarray(bk[sl]),
                "bv": np.ascontiguousarray(bv[sl]),
            }
        )
    return in_maps


def run(inputs, **kwargs):
    """Run the kernel; returns (full_output, BassKernelResults)."""
    from concourse.bass_utils import run_bass_kernel_spmd

    kwargs.pop("debug", None)
    nc = _get_nc()
    in_maps = _make_in_maps(inputs)
    res = run_bass_kernel_spmd(nc, in_maps, core_ids=list(range(NCORES)), **kwargs)
    bo = np.asarray(inputs["bo"], dtype=np.float32)
    final = np.empty((B, S, D), np.float32)
    for b in range(B):
        final[b] = (
            res.results[2 * b]["out"].astype(np.float32)
            + res.results[2 * b + 1]["out"].astype(np.float32)
            + bo
        )
    return final, res


def kernel(**inputs):
    return run(inputs)[0]


# revision 12
# speedup vs baseline: 1.0866x; 1.0072x over previous
"""Multi-head attention (B=4, S=2048, D=1024, H=16) on 8 TRN2 NeuronCores.

Sharding (Megatron-style, per spec hint): data-parallel over batch (4) x
tensor-parallel over heads (2 groups of 8). Core c handles batch c//2,
head-group c%2. QKV projections column-sharded, output projection
row-sharded; the two partial outputs per batch are summed on the host
together with the output bias.

Per-core kernel (one NeuronCore, 8 heads, 2048 tokens), v2:
  - Scores transposed ST[k, q] with softmax-exp (no max subtraction) as one
    ACT pass per [128, 2, 512] score tile, bf16 out.
  - att@V uses the probabilities as the STATIONARY operand ([128k, 128q]
    slices) and v tiles [128k, 64] as moving, so the output [128q, 64]
    fills all 128 PSUM partitions: half the PE cost of the v-stationary
    form. A head-pair's whole output (4 q-tiles x 2 heads x 64) packs into
    exactly one PSUM bank with a single accumulation start/stop.
  - The softmax denominator Z accumulates via 1-column matmuls against the
    v_aug ones column into a separate z bank.
  - Normalration is one DVE pass per pair (stride-0 broadcast of 1/Z);
    the normalized [q, feature] tiles are transposed back to feature-major
    by the DMA xbar (dma_start_transpose), not the PE.
  - att@V chains are spliced into the NEXT pair's score loop (PE slack per
    kt2 slot), so ScalarE streams exps with few gaps; k/v/q projections and
    the previous group's output projection are spliced the same way.
"""

import sys

if "/opt/trn_rl_repo" not in sys.path:
    sys.path.insert(0, "/opt/trn_rl_repo")

import numpy as np

B, S, D = 4, 2048, 1024
H, DK = 16, 64
NCORES = 8
HC = H // 2            # heads per core
DC = HC * DK           # 512 local features per core
INV_SCALE = 1.0 / 8.0  # 1/sqrt(DK)
P = 128
NDCH = D // P          # 8 contraction chunks for projections
NFC = DC // P          # 4 local feature chunks
NKT = S // P           # 16 key tiles
NQG = 4                # query groups
QG = S // NQG          # 512 queries per group
NQT = QG // P          # 4 query tiles per group
VW = DK + 1            # 65: v columns + ones column
NHP = HC // 2          # head pairs

_CACHE = {}


def _build():
    import concourse.bass as bass
    import concourse.bacc as bacc
    import concourse.tile as tile
    import concourse.mybir as mybir
    from concourse.bass import ts, ds

    f32 = mybir.dt.float32
    f32r = mybir.dt.float32r
    bf16 = mybir.dt.bfloat16
    AF = mybir.ActivationFunctionType
    ALU = mybir.AluOpType

    nc = bacc.Bacc("TRN2", target_bir_lowering=False, num_devices=NCORES)

    xqT = nc.dram_tensor("xqT", [D, S], bf16, kind="ExternalInput")
    xkT = nc.dram_tensor("xkT", [D, S], bf16, kind="ExternalInput")
    xvT = nc.dram_tensor("xvT", [D, S], bf16, kind="ExternalInput")
    wq = nc.dram_tensor("wq", [D, DC], bf16, kind="ExternalInput")
    wk = nc.dram_tensor("wk", [D, DC], bf16, kind="ExternalInput")
    wv = nc.dram_tensor("wv", [D, DC], bf16, kind="ExternalInput")
    wo = nc.dram_tensor("wo", [DC, D], bf16, kind="ExternalInput")
    bq = nc.dram_tensor("bq", [DC], f32, kind="ExternalInput")
    bk = nc.dram_tensor("bk", [DC], f32, kind="ExternalInput")
    bv = nc.dram_tensor("bv", [DC], f32, kind="ExternalInput")
    out = nc.dram_tensor("out", [S, D], bf16, kind="ExternalOutput")

    with tile.TileContext(nc) as tc:
        with (
            tc.tile_pool(name="persist", bufs=1) as persist,
            tc.tile_pool(name="wts", bufs=2) as wpool,
            tc.tile_pool(name="xin", bufs=4) as xpool,
            tc.tile_pool(name="qt", bufs=2) as qpool,
            tc.tile_pool(name="expst", bufs=18) as epool,
            tc.tile_pool(name="osb", bufs=2) as ospool,
            tc.tile_pool(name="att", bufs=1) as atpool,
            tc.tile_pool(name="small", bufs=2) as spool,
            tc.tile_pool(name="oc", bufs=2) as ocpool,
            tc.tile_pool(name="pp", bufs=2, space="PSUM") as pp,
            tc.tile_pool(name="st", bufs=2, space="PSUM") as st_pool,
            tc.tile_pool(name="av", bufs=1, space="PSUM") as avp,
            tc.tile_pool(name="zp", bufs=1, space="PSUM") as zpool,
        ):
            # ---- persistent SBUF tensors ----
            kT = persist.tile([P, NFC, S], bf16)          # 16KB/part
            v_aug = persist.tile([P, NKT, HC, VW], bf16)  # ~16.6KB/part
            wo_sb = persist.tile([P, NFC, D], bf16)       # 8KB/part
            bq_sb = persist.tile([P, NFC], f32)
            bk_sb = persist.tile([P, NFC], f32)
            bvb = persist.tile([P, DC], f32)              # bias_v broadcast

            nc.sync.dma_start(out=bq_sb, in_=bq.rearrange("(c p) -> p c", p=P))
            nc.sync.dma_start(out=bk_sb, in_=bk.rearrange("(c p) -> p c", p=P))
            bv_ap = bv.ap()
            bvb_src = bass.AP(
                tensor=bv_ap.tensor, offset=bv_ap.offset, ap=[[0, P], *bv_ap.ap]
            )
            nc.sync.dma_start(out=bvb, in_=bvb_src)
            # ones column of v_aug (softmax denominator trick)
            ones_st = persist.tile([P, P], f32)
            nc.vector.memset(ones_st, 1.0)
            nc.vector.tensor_copy(
                out=v_aug[:, :, :, DK],
                in_=ones_st.rearrange("p (k h) -> p k h", k=NKT),
            )

            # ---- emission helpers (PE program order == emission order) ----
            def load_w(w_dram, name, tag="w", bufs=None, fc_split=False,
                       defer=False):
                w_sb = wpool.tile([P, NDCH, DC], bf16, tag=tag, name=name, bufs=bufs)
                wr = w_dram.rearrange("(c p) f -> p c f", p=P)
                if fc_split:
                    # first half of the feature chunks now; rest via thunk
                    nc.sync.dma_start(out=w_sb[:, :, 0:DC // 2], in_=wr[:, :, 0:DC // 2])
                    rest = lambda: nc.sync.dma_start(
                        out=w_sb[:, :, DC // 2:], in_=wr[:, :, DC // 2:])
                    if defer:
                        return w_sb, rest
                    rest()
                else:
                    nc.sync.dma_start(out=w_sb, in_=wr)
                return w_sb

            def load_x(xT_dram, g, name, tag="x", bufs=None, split=False):
                x_sb = xpool.tile([P, NDCH, QG], bf16, tag=tag, name=name, bufs=bufs)
                xr = xT_dram.rearrange("(c p) t -> p c t", p=P)[:, :, ts(g, QG)]
                if split:
                    h_ = NDCH // 2
                    nc.sync.dma_start(out=x_sb[:, 0:h_, :], in_=xr[:, 0:h_, :])
                    nc.sync.dma_start(out=x_sb[:, h_:, :], in_=xr[:, h_:, :])
                else:
                    nc.sync.dma_start(out=x_sb, in_=xr)
                return x_sb

            def kproj_chain(w_sb, x_sb, g, fc, half=None, state={}):
                if half in (None, 0):
                    state["ps"] = pp.tile(
                        [P, QG], f32, tag="pp", name=f"pk_{g}_{fc}"
                    )
                ps = state["ps"]
                lo = 0 if half in (None, 0) else NDCH // 2
                hi = NDCH if half in (None, 1) else NDCH // 2
                for dch in range(lo, hi):
                    nc.tensor.matmul(
                        ps, w_sb[:, dch, ts(fc, P)], x_sb[:, dch, :],
                        start=(dch == 0), stop=(dch == NDCH - 1),
                    )
                if half in (None, 1):
                    nc.vector.tensor_scalar(
                        out=kT[:, fc, ts(g, QG)], in0=ps,
                        scalar1=bk_sb[:, fc : fc + 1], scalar2=None, op0=ALU.add,
                    )

            def qproj_chain(w_sb, x_sb, qT, g, fc, half=None, state={}):
                """half=0: first 4 contraction chunks (new psum tile);
                half=1: last 4 + bias; None: whole chain."""
                if half in (None, 0):
                    state["ps"] = pp.tile(
                        [P, QG], f32, tag="pp", name=f"pq_{g}_{fc}"
                    )
                ps = state["ps"]
                lo = 0 if half in (None, 0) else NDCH // 2
                hi = NDCH if half in (None, 1) else NDCH // 2
                for dch in range(lo, hi):
                    nc.tensor.matmul(
                        ps, w_sb[:, dch, ts(fc, P)], x_sb[:, dch, :],
                        start=(dch == 0), stop=(dch == NDCH - 1),
                    )
                if half in (None, 1):
                    nc.vector.tensor_scalar(
                        out=qT[:, fc, :], in0=ps,
                        scalar1=bq_sb[:, fc : fc + 1], scalar2=None, op0=ALU.add,
                    )

            def vproj_tile(w_sb, x_sb, kt):
                tt = kt % NQT
                ps = pp.tile([P, DC], f32, tag="pp", name=f"pv_{kt}")
                for dch in range(NDCH):
                    nc.tensor.matmul(
                        ps, x_sb[:, dch, ts(tt, P)], w_sb[:, dch, :],
                        start=(dch == 0), stop=(dch == NDCH - 1),
                    )
                nc.vector.tensor_add(
                    out=v_aug[:, kt, :, 0:DK],
                    in0=ps.rearrange("p (h d) -> p h d", h=HC),
                    in1=bvb.rearrange("p (h d) -> p h d", h=HC),
                )

            def outproj_chain(attnT, g, tt, eg, pool=None):
                pool = pool or pp
                ps = pool.tile(
                    [P, DC], f32, tag="pp" if pool is pp else "av",
                    name=f"po_{g}_{tt}_{eg}",
                )
                for fc in range(NFC):
                    nc.tensor.matmul(
                        ps, attnT[:, fc, ts(tt, P)], wo_sb[:, fc, ts(eg, DC)],
                        start=(fc == 0), stop=(fc == NFC - 1),
                    )
                o_sb = ocpool.tile([P, DC], bf16, tag="osb", name=f"ob_{g}_{tt}_{eg}")
                nc.vector.tensor_copy(out=o_sb, in_=ps)
                nc.sync.dma_start(
                    out=out[ds(g * QG + tt * P, P), ts(eg, DC)], in_=o_sb
                )

            # ---- pair state: est tiles + av/z banks, consumed one pair later
            class PairState:
                def __init__(self, g, hp):
                    self.g, self.hp = g, hp
                    self.ests = {}   # h -> list of 8 est tiles [P, 2, QG]
                    self.av = None   # [P, NQT, 2, DK] f32 psum (1 bank)
                    self.zt = None   # [P, QG] f32 psum (1 bank; cols 0:8 used)

            def attv_slice(ps_, s):
                """att@V + Z matmuls consuming est[s] (key tiles 2s, 2s+1)."""
                g, hp = ps_.g, ps_.hp
                if s == 0:
                    ps_.av = avp.tile(
                        [P, NQT, 2, DK], f32, tag="av", name=f"av_{g}_{hp}"
                    )
                    ps_.zt = zpool.tile([P, QG], f32, tag="z", name=f"z_{g}_{hp}")
                last = NKT // 2 - 1
                for kk in range(2):
                    kt = 2 * s + kk
                    for qt in range(NQT):
                        for hh in range(2):
                            h = 2 * hp + hh
                            est = ps_.ests[h][s]
                            stat = est[:, kk, ts(qt, P)]
                            first = s == 0 and kk == 0 and qt == 0 and hh == 0
                            lastm = s == last and kk == 1 and qt == NQT - 1 and hh == 1
                            nc.tensor.matmul(
                                ps_.av[:, qt, hh, :], stat,
                                v_aug[:, kt, h, 0:DK],
                                start=first, stop=lastm,
                            )
                            c = qt * 2 + hh
                            nc.tensor.matmul(
                                ps_.zt[:, c : c + 1], stat,
                                v_aug[:, kt, h, DK:VW],
                                start=first, stop=lastm,
                            )

            def finish_pair(ps_, o_sb_tiles, qts=None):
                """reciprocal + normalize for a finished pair.

                qts: restrict the normalize to these q-tiles (tail
                pipelining); reciprocal runs only when qts is None or
                starts at qt 0."""
                g, hp = ps_.g, ps_.hp
                if qts is None or qts[0] == 0:
                    ps_.rz = spool.tile(
                        [P, NQT, 2], f32r, tag="rz", name=f"rz_{g}_{hp}"
                    )
                    with nc.allow_low_precision("softmax denom reciprocal"):
                        nc.vector.reciprocal(
                            out=ps_.rz,
                            in_=ps_.zt[:, 0 : 2 * NQT].rearrange(
                                "p (q h) -> p q h", q=NQT
                            ),
                        )
                o_sb = o_sb_tiles[g]
                if qts is None:
                    nc.vector.tensor_tensor(
                        out=o_sb[:, :, 2 * hp : 2 * hp + 2, :],
                        in0=ps_.av,
                        in1=ps_.rz.unsqueeze(-1).broadcast_to([P, NQT, 2, DK]),
                        op=ALU.mult,
                    )
                else:
                    for qt in qts:
                        nc.vector.tensor_tensor(
                            out=o_sb[:, qt, 2 * hp : 2 * hp + 2, :],
                            in0=ps_.av[:, qt, :, :],
                            in1=ps_.rz[:, qt, :].unsqueeze(-1).broadcast_to(
                                [P, 2, DK]),
                            op=ALU.mult,
                        )

            def transposes(g, o_sb_tiles, attnT):
                o_sb = o_sb_tiles[g]
                for qt in range(NQT):
                    for fc in range(NFC):
                        nc.sync.dma_start_transpose(
                            out=attnT[:, fc, ts(qt, P)],
                            in_=o_sb[:, qt, 2 * fc : 2 * fc + 2, :],
                        )

            # =========== prelude ===========
            # DMA order tuned so the first-score chain (wk fc01, xk0, wq
            # fc01, xq0) clears in ~10us and fill-phase consumers (xv0, wv,
            # xk1-3) arrive before their spliced chains need them.
            wk_sb, wk_rest = load_w(wk, "w_k", fc_split=True, defer=True)
            xk_sbs = [load_x(xkT, 0, "x_k_0", tag="xk", bufs=4, split=True)]
            wq_sb, wq_rest = load_w(wq, "w_q", tag="wq", bufs=1, fc_split=True,
                                    defer=True)
            xq_tiles = {0: load_x(xqT, 0, "x_q_0", tag="xq", bufs=2, split=True)}
            kproj_chain(wk_sb, xk_sbs[0], 0, 0, half=0)
            kproj_chain(wk_sb, xk_sbs[0], 0, 0, half=1)

            qst = {0: qpool.tile([P, NFC, QG], bf16, tag="qT", name="qT_0")}
            qproj_chain(wq_sb, xq_tiles[0], qst[0], 0, 0, half=0)
            qproj_chain(wq_sb, xq_tiles[0], qst[0], 0, 0, half=1)

            xk_sbs.append(load_x(xkT, 1, "x_k_1", tag="xk", bufs=4))
            wv_sb = load_w(wv, "w_v")
            xv_tiles = {0: load_x(xvT, 0, "x_v_0", tag="xv", bufs=2)}
            xk_sbs.append(load_x(xkT, 2, "x_k_2", tag="xk", bufs=4))
            xk_sbs.append(load_x(xkT, 3, "x_k_3", tag="xk", bufs=4))
            wk_rest()
            wq_rest()
            nc.sync.dma_start(out=wo_sb, in_=wo.rearrange("(c p) e -> p c e", p=P))

            # =========== splice schedule ===========
            # pair index p = 4*g + hp runs score loop slots 0..7; sched[p][s]
            # is a list of thunks emitted before slot s's score matmuls.
            sched = {p: {s: [] for s in range(8)} for p in range(16)}

            def at(p, s, fn):
                sched[p][s].append(fn)

            # kproj: fc=0 for kg>=1 early in pair 0; fc=f in pair f-1... but
            # pair (0,hp) reads kT chunk hp for all kt: chunk fc must be fully
            # projected (all 4 kg) before pair (0,fc) starts.
            for kg, s_ in [(1, 0), (2, 1), (3, 3)]:
                at(0, s_, lambda kg=kg: kproj_chain(wk_sb, xk_sbs[kg], kg, 0))
            for fc in range(1, 4):
                for kg in range(4):
                    at(fc - 1, 2 * kg + 1, lambda kg=kg, fc=fc: kproj_chain(
                        wk_sb, xk_sbs[kg], kg, fc))
            # vproj: 10 tiles in pair 0 (extra on later slots), 6 in pair 1;
            # v_aug[kt] needed by attV(0,0) slice s=kt//2 at pair 1 slot s.
            # xv loads run >=2 slots ahead of their first vproj consumer.
            for vg, (p_, s_) in {1: (0, 1), 2: (0, 5), 3: (0, 7)}.items():
                at(p_, s_, lambda vg=vg: xv_tiles.__setitem__(
                    vg, load_x(xvT, vg, f"x_v_{vg}", tag="xv", bufs=2)))
            vq = [(0, 0, 1), (0, 1, 1), (0, 2, 1), (0, 3, 1), (0, 4, 2),
                  (0, 5, 2), (0, 6, 2), (0, 7, 2), (1, 0, 2), (1, 1, 2),
                  (1, 2, 2)]
            kt_next = 0
            for p_, s_, n_ in vq:
                for _ in range(n_):
                    if kt_next >= NKT:
                        break
                    kt = kt_next
                    kt_next += 1
                    at(p_, s_, lambda kt=kt: vproj_tile(
                        wv_sb, xv_tiles[kt // NQT], kt))
            # qproj for pair p+1 at pair p slot 5 (+ xq loads 2 pairs early)
            for p in range(15):
                g1, fc1 = divmod(p + 1, 4)
                if fc1 == 0 and g1 > 0:
                    at(p - 2 if p >= 2 else 0, 1, lambda g1=g1: xq_tiles.__setitem__(
                        g1, load_x(xqT, g1, f"x_q_{g1}", tag="xq", bufs=2)))
                    at(p, 5, lambda g1=g1: (
                        qst.__setitem__(g1, qpool.tile(
                            [P, NFC, QG], bf16, tag="qT", name=f"qT_{g1}")),
                        qproj_chain(wq_sb, xq_tiles[g1], qst[g1], g1, 0,
                                    half=0))[-1])
                    at(p, 7, lambda g1=g1: qproj_chain(
                        wq_sb, xq_tiles[g1], qst[g1], g1, 0, half=1))
                else:
                    at(p, 5, lambda g1=g1, fc1=fc1: qproj_chain(
                        wq_sb, xq_tiles[g1], qst[g1], g1, fc1, half=0))
                    at(p, 7, lambda g1=g1, fc1=fc1: qproj_chain(
                        wq_sb, xq_tiles[g1], qst[g1], g1, fc1, half=1))
            # outproj(g) chains spliced into pairs of group g+1
            op_slots = [(1, 4), (1, 6), (2, 2), (2, 4), (2, 6), (3, 2),
                        (3, 4), (3, 6)]
            attnT_holder = {}
            for g in range(3):
                for i, (hp_, s_) in enumerate(op_slots):
                    tt, eg = divmod(i, 2)
                    at(4 * (g + 1) + hp_, s_, lambda g=g, tt=tt, eg=eg: outproj_chain(
                        attnT_holder[g], g, tt, eg))

            # =========== main loop ===========
            o_sb_tiles = {}
            prev_pair = None   # PairState consumed by current pair's splices
            done_pair = None   # PairState whose attV completed last pair
            # (its finish_pair runs at the START of this pair so the DVE
            # queue never parks on unmet deps — DVE is in-order)

            for p in range(16):
                g, hp = divmod(p, 4)
                if g not in o_sb_tiles:
                    o_sb_tiles[g] = ospool.tile(
                        [P, NQT, HC, DK], bf16, tag="osb2", name=f"o_{g}"
                    )
                cur = PairState(g, hp)
                qT = qst[g]
                for kt2 in range(NKT // 2):
                    if kt2 == 0 and done_pair is not None:
                        finish_pair(done_pair, o_sb_tiles)
                        if done_pair.hp == NHP - 1:
                            gg = done_pair.g
                            attnT_holder[gg] = atpool.tile(
                                [P, NFC, QG], bf16, tag="attnT", name=f"aT_{gg}"
                            )
                            transposes(gg, o_sb_tiles, attnT_holder[gg])
                        done_pair = None
                    def emit_splices():
                        if prev_pair is not None:
                            attv_slice(prev_pair, kt2)
                        for fn in sched[p][kt2]:
                            fn()

                    def emit_scores():
                        sts = {}
                        for hh in range(2):
                            h = 2 * hp + hh
                            sts[h] = st_pool.tile(
                                [P, 2, QG], f32, tag="st",
                                name=f"st_{g}_{h}_{kt2}"
                            )
                        for kk in range(2):
                            kt = 2 * kt2 + kk
                            for hh in range(2):
                                h = 2 * hp + hh
                                r0 = hh * DK
                                nc.tensor.matmul(
                                    sts[h][:, kk, :],
                                    kT[r0 : r0 + DK, hp, ts(kt, P)],
                                    qT[r0 : r0 + DK, hp, :],
                                    start=True, stop=True,
                                    tile_position=(r0, 0),
                                )
                        for hh in range(2):
                            h = 2 * hp + hh
                            e = epool.tile(
                                [P, 2, QG], bf16, tag="est",
                                name=f"est_{g}_{h}_{kt2}"
                            )
                            cur.ests.setdefault(h, []).append(e)
                            nc.scalar.activation(
                                out=e, in_=sts[h], func=AF.Exp, scale=INV_SCALE
                            )

                    # fill phase (pairs 0-2): ACT is starved, so feed it
                    # scores before the heavy projection splices; steady
                    # state: splices first (PE uses the st-ring wait time)
                    if p < 3:
                        emit_scores()
                        emit_splices()
                    else:
                        emit_splices()
                        emit_scores()
                # previous pair's attV is complete; finish it at the start
                # of the next pair (deps met there, no DVE queue parking)
                done_pair = prev_pair
                prev_pair = cur

            # =========== tail: last pair's attV + outproj of group 3 ====
            # per-qt pipelining: as soon as qt's normalize lands, its
            # transposes, outproj chains and output DMA flow while the PE
            # works the next qt.
            finish_pair(done_pair, o_sb_tiles)
            for s in range(NKT // 2):
                attv_slice(prev_pair, s)
            attnT_holder[3] = atpool.tile(
                [P, NFC, QG], bf16, tag="attnT", name="aT_3"
            )
            o_sb3 = o_sb_tiles[3]
            for qt in range(NQT):
                finish_pair(prev_pair, o_sb_tiles, qts=[qt])
                for fc in range(NFC):
                    nc.sync.dma_start_transpose(
                        out=attnT_holder[3][:, fc, ts(qt, P)],
                        in_=o_sb3[:, qt, 2 * fc : 2 * fc + 2, :],
                    )
                for eg in range(2):
                    outproj_chain(attnT_holder[3], 3, qt, eg)

    nc.compile()
    return nc


def _get_nc(debug=False):
    if "nc" not in _CACHE:
        _CACHE["nc"] = _build()
    return _CACHE["nc"]


def _tf32(a):
    """Round fp32 to the TF32 grid (10-bit mantissa, round-to-nearest-even)."""
    u = np.ascontiguousarray(a, dtype=np.float32).view(np.uint32)
    u = (u + np.uint32(0xFFF) + ((u >> np.uint32(13)) & np.uint32(1))) & np.uint32(
        0xFFFFE000
    )
    return u.view(np.float32)


def _bf16(a):
    import ml_dtypes

    return np.ascontiguousarray(a, dtype=np.float32).astype(ml_dtypes.bfloat16)


def _make_in_maps(inputs):
    q = np.asarray(inputs["query"], dtype=np.float32)
    k = np.asarray(inputs["key"], dtype=np.float32)
    v = np.asarray(inputs["value"], dtype=np.float32)
    wq = np.asarray(inputs["wq"], dtype=np.float32)
    wk = np.asarray(inputs["wk"], dtype=np.float32)
    wv = np.asarray(inputs["wv"], dtype=np.float32)
    wo = np.asarray(inputs["wo"], dtype=np.float32)
    bq = np.asarray(inputs["bq"], dtype=np.float32)
    bk = np.asarray(inputs["bk"], dtype=np.float32)
    bv = np.asarray(inputs["bv"], dtype=np.float32)

    xT = [(_bf16(q[b].T), _bf16(k[b].T), _bf16(v[b].T)) for b in range(B)]
    in_maps = []
    for c in range(NCORES):
        b, g = divmod(c, 2)
        sl = slice(g * DC, (g + 1) * DC)
        in_maps.append(
            {
                "xqT": xT[b][0],
                "xkT": xT[b][1],
                "xvT": xT[b][2],
                "wq": _bf16(wq[:, sl]),
                "wk": _bf16(wk[:, sl]),
                "wv": _bf16(wv[:, sl]),
                "wo": _bf16(wo[sl, :]),
                "bq": np.ascontiguousarray(bq[sl]),
                "bk": np.ascontiguousarray(bk[sl]),
                "bv": np.ascontiguousarray(bv[sl]),
            }
        )
    return in_maps


def run(inputs, **kwargs):
    """Run the kernel; returns (full_output, BassKernelResults)."""
    from concourse.bass_utils import run_bass_kernel_spmd

    kwargs.pop("debug", None)
    nc = _get_nc()
    in_maps = _make_in_maps(inputs)
    res = run_bass_kernel_spmd(nc, in_maps, core_ids=list(range(NCORES)), **kwargs)
    bo = np.asarray(inputs["bo"], dtype=np.float32)
    final = np.empty((B, S, D), np.float32)
    for b in range(B):
        final[b] = (
            res.results[2 * b]["out"].astype(np.float32)
            + res.results[2 * b + 1]["out"].astype(np.float32)
            + bo
        )
    return final, res


def kernel(**inputs):
    return run(inputs)[0]


# revision 13
# speedup vs baseline: 1.1086x; 1.0202x over previous
"""Multi-head attention (B=4, S=2048, D=1024, H=16) on 8 TRN2 NeuronCores.

Sharding (Megatron-style, per spec hint): data-parallel over batch (4) x
tensor-parallel over heads (2 groups of 8). Core c handles batch c//2,
head-group c%2. QKV projections column-sharded, output projection
row-sharded; the two partial outputs per batch are summed on the host
together with the output bias.

Per-core kernel (one NeuronCore, 8 heads, 2048 tokens), v2:
  - Scores transposed ST[k, q] with softmax-exp (no max subtraction) as one
    ACT pass per [128, 2, 512] score tile, bf16 out.
  - att@V uses the probabilities as the STATIONARY operand ([128k, 128q]
    slices) and v tiles [128k, 64] as moving, so the output [128q, 64]
    fills all 128 PSUM partitions: half the PE cost of the v-stationary
    form. A head-pair's whole output (4 q-tiles x 2 heads x 64) packs into
    exactly one PSUM bank with a single accumulation start/stop.
  - The softmax denominator Z accumulates via 1-column matmuls against the
    v_aug ones column into a separate z bank.
  - Normalration is one DVE pass per pair (stride-0 broadcast of 1/Z);
    the normalized [q, feature] tiles are transposed back to feature-major
    by the DMA xbar (dma_start_transpose), not the PE.
  - att@V chains are spliced into the NEXT pair's score loop (PE slack per
    kt2 slot), so ScalarE streams exps with few gaps; k/v/q projections and
    the previous group's output projection are spliced the same way.
"""

import sys

if "/opt/trn_rl_repo" not in sys.path:
    sys.path.insert(0, "/opt/trn_rl_repo")

import numpy as np

B, S, D = 4, 2048, 1024
H, DK = 16, 64
NCORES = 8
HC = H // 2            # heads per core
DC = HC * DK           # 512 local features per core
INV_SCALE = 1.0 / 8.0 / (32.0 * 32.0)  # 1/sqrt(DK), /32^2 fp8 weight scale
P = 128
NDCH = D // P          # 8 contraction chunks for projections
NFC = DC // P          # 4 local feature chunks
NKT = S // P           # 16 key tiles
NQG = 4                # query groups
QG = S // NQG          # 512 queries per group
NQT = QG // P          # 4 query tiles per group
VW = DK + 1            # 65: v columns + ones column
NHP = HC // 2          # head pairs

_CACHE = {}


def _build():
    import concourse.bass as bass
    import concourse.bacc as bacc
    import concourse.tile as tile
    import concourse.mybir as mybir
    from concourse.bass import ts, ds

    f32 = mybir.dt.float32
    f32r = mybir.dt.float32r
    bf16 = mybir.dt.bfloat16
    AF = mybir.ActivationFunctionType
    ALU = mybir.AluOpType

    nc = bacc.Bacc("TRN2", target_bir_lowering=False, num_devices=NCORES)

    f8 = mybir.dt.float8e4
    DR = mybir.MatmulPerfMode.DoubleRow
    xqT = (nc.dram_tensor("xq8", [D, S], f8, kind="ExternalInput"),
           nc.dram_tensor("xq8l", [D, S], f8, kind="ExternalInput"))
    xkT = (nc.dram_tensor("xk8", [D, S], f8, kind="ExternalInput"),
           nc.dram_tensor("xk8l", [D, S], f8, kind="ExternalInput"))
    xvT = (nc.dram_tensor("xv8", [D, S], f8, kind="ExternalInput"),
           nc.dram_tensor("xv8l", [D, S], f8, kind="ExternalInput"))
    wq = (nc.dram_tensor("wq8", [D, DC], f8, kind="ExternalInput"),
          nc.dram_tensor("wq8l", [D, DC], f8, kind="ExternalInput"))
    wk = (nc.dram_tensor("wk8", [D, DC], f8, kind="ExternalInput"),
          nc.dram_tensor("wk8l", [D, DC], f8, kind="ExternalInput"))
    wv = (nc.dram_tensor("wv8", [D, DC], f8, kind="ExternalInput"),
          nc.dram_tensor("wv8l", [D, DC], f8, kind="ExternalInput"))
    wo = nc.dram_tensor("wo", [DC, D], bf16, kind="ExternalInput")
    bq = nc.dram_tensor("bq", [DC], f32, kind="ExternalInput")
    bk = nc.dram_tensor("bk", [DC], f32, kind="ExternalInput")
    bv = nc.dram_tensor("bv", [DC], f32, kind="ExternalInput")
    out = nc.dram_tensor("out", [S, D], bf16, kind="ExternalOutput")

    with tile.TileContext(nc) as tc:
        with (
            tc.tile_pool(name="persist", bufs=1) as persist,
            tc.tile_pool(name="wts", bufs=2) as wpool,
            tc.tile_pool(name="xin", bufs=4) as xpool,
            tc.tile_pool(name="qt", bufs=2) as qpool,
            tc.tile_pool(name="expst", bufs=18) as epool,
            tc.tile_pool(name="osb", bufs=2) as ospool,
            tc.tile_pool(name="att", bufs=1) as atpool,
            tc.tile_pool(name="small", bufs=2) as spool,
            tc.tile_pool(name="oc", bufs=2) as ocpool,
            tc.tile_pool(name="pp", bufs=2, space="PSUM") as pp,
            tc.tile_pool(name="st", bufs=2, space="PSUM") as st_pool,
            tc.tile_pool(name="av", bufs=1, space="PSUM") as avp,
            tc.tile_pool(name="zp", bufs=1, space="PSUM") as zpool,
        ):
            # ---- persistent SBUF tensors ----
            kT = persist.tile([P, NFC, S], bf16)          # 16KB/part
            v_aug = persist.tile([P, NKT, HC, VW], bf16)  # ~16.6KB/part
            wo_sb = persist.tile([P, NFC, D], bf16)       # 8KB/part
            bq_sb = persist.tile([P, NFC], f32)
            bk_sb = persist.tile([P, NFC], f32)
            bvb = persist.tile([P, DC], f32)              # bias_v broadcast

            nc.sync.dma_start(out=bq_sb, in_=bq.rearrange("(c p) -> p c", p=P))
            nc.sync.dma_start(out=bk_sb, in_=bk.rearrange("(c p) -> p c", p=P))
            bv_ap = bv.ap()
            bvb_src = bass.AP(
                tensor=bv_ap.tensor, offset=bv_ap.offset, ap=[[0, P], *bv_ap.ap]
            )
            nc.sync.dma_start(out=bvb, in_=bvb_src)
            # ones column of v_aug (softmax denominator trick)
            ones_st = persist.tile([P, P], f32)
            nc.vector.memset(ones_st, 32.0)
            nc.vector.tensor_copy(
                out=v_aug[:, :, :, DK],
                in_=ones_st.rearrange("p (k h) -> p k h", k=NKT),
            )

            # ---- emission helpers (PE program order == emission order) ----
            def load_w(w_dram, name, tag="w", bufs=None, fc_split=False,
                       defer=False):
                pair = []
                for i, wd in enumerate(w_dram):
                    pair.append(wpool.tile(
                        [P, NDCH, DC], f8, tag=f"{tag}{i}", name=f"{name}_{i}",
                        bufs=bufs))
                rests = []
                for w_sb, wd in zip(pair, w_dram):
                    wr = wd.rearrange("(c p) f -> p c f", p=P)
                    if fc_split:
                        nc.sync.dma_start(
                            out=w_sb[:, :, 0:DC // 2], in_=wr[:, :, 0:DC // 2])
                        rests.append(lambda w_sb=w_sb, wr=wr: nc.sync.dma_start(
                            out=w_sb[:, :, DC // 2:], in_=wr[:, :, DC // 2:]))
                    else:
                        nc.sync.dma_start(out=w_sb, in_=wr)
                if fc_split:
                    rest = lambda: [r() for r in rests]
                    if defer:
                        return tuple(pair), rest
                    rest()
                return tuple(pair)

            def load_x(xT_dram, g, name, tag="x", bufs=None, split=False):
                pair = []
                for i, xd in enumerate(xT_dram):
                    x_sb = xpool.tile(
                        [P, NDCH, QG], f8, tag=f"{tag}{i}", name=f"{name}_{i}",
                        bufs=bufs)
                    pair.append(x_sb)
                    xr = xd.rearrange("(c p) t -> p c t", p=P)[:, :, ts(g, QG)]
                    if split:
                        h_ = NDCH // 2
                        nc.sync.dma_start(out=x_sb[:, 0:h_, :], in_=xr[:, 0:h_, :])
                        nc.sync.dma_start(out=x_sb[:, h_:, :], in_=xr[:, h_:, :])
                    else:
                        nc.sync.dma_start(out=x_sb, in_=xr)
                return tuple(pair)

            def proj_mms(ps, w_pair, x_pair, fc, half):
                """3-term hi/lo fp8 DoubleRow chain: w8*x8 + w8*x8l + w8l*x8.
                Contraction pairs c of 256 rows; 3 DR matmuls each."""
                w8, w8l = w_pair
                x8, x8l = x_pair
                cs = range(0, NDCH // 4) if half == 0 else (
                    range(NDCH // 4, NDCH // 2) if half == 1
                    else range(NDCH // 2))
                ncp = NDCH // 2
                for c in cs:
                    d = slice(2 * c, 2 * c + 2)
                    for t, (wt, xt) in enumerate(
                        ((w8, x8), (w8, x8l), (w8l, x8))
                    ):
                        nc.tensor.matmul(
                            ps, wt[:, d, ts(fc, P)], xt[:, d, :],
                            start=(c == 0 and t == 0),
                            stop=(c == ncp - 1 and t == 2),
                            perf_mode=DR,
                        )

            def kproj_chain(w_sb, x_sb, g, fc, half=None, state={}):
                if half in (None, 0):
                    state["ps"] = pp.tile(
                        [P, QG], f32, tag="pp", name=f"pk_{g}_{fc}"
                    )
                ps = state["ps"]
                proj_mms(ps, w_sb, x_sb, fc, half)
                if half in (None, 1):
                    nc.vector.tensor_scalar(
                        out=kT[:, fc, ts(g, QG)], in0=ps,
                        scalar1=bk_sb[:, fc : fc + 1], scalar2=None, op0=ALU.add,
                    )

            def qproj_chain(w_sb, x_sb, qT, g, fc, half=None, state={}):
                if half in (None, 0):
                    state["ps"] = pp.tile(
                        [P, QG], f32, tag="pp", name=f"pq_{g}_{fc}"
                    )
                ps = state["ps"]
                proj_mms(ps, w_sb, x_sb, fc, half)
                if half in (None, 1):
                    nc.vector.tensor_scalar(
                        out=qT[:, fc, :], in0=ps,
                        scalar1=bq_sb[:, fc : fc + 1], scalar2=None, op0=ALU.add,
                    )

            def vproj_tile(w_sb, x_sb, kt):
                tt = kt % NQT
                w8, w8l = w_sb
                x8, x8l = x_sb
                ps = pp.tile([P, DC], f32, tag="pp", name=f"pv_{kt}")
                ncp = NDCH // 2
                for c in range(ncp):
                    d = slice(2 * c, 2 * c + 2)
                    for t, (xt, wt) in enumerate(
                        ((x8, w8), (x8, w8l), (x8l, w8))
                    ):
                        nc.tensor.matmul(
                            ps, xt[:, d, ts(tt, P)], wt[:, d, :],
                            start=(c == 0 and t == 0),
                            stop=(c == ncp - 1 and t == 2),
                            perf_mode=DR,
                        )
                nc.vector.tensor_add(
                    out=v_aug[:, kt, :, 0:DK],
                    in0=ps.rearrange("p (h d) -> p h d", h=HC),
                    in1=bvb.rearrange("p (h d) -> p h d", h=HC),
                )

            def outproj_chain(attnT, g, tt, eg, pool=None):
                pool = pool or pp
                ps = pool.tile(
                    [P, DC], f32, tag="pp" if pool is pp else "av",
                    name=f"po_{g}_{tt}_{eg}",
                )
                for fc in range(NFC):
                    nc.tensor.matmul(
                        ps, attnT[:, fc, ts(tt, P)], wo_sb[:, fc, ts(eg, DC)],
                        start=(fc == 0), stop=(fc == NFC - 1),
                    )
                o_sb = ocpool.tile([P, DC], bf16, tag="osb", name=f"ob_{g}_{tt}_{eg}")
                nc.vector.tensor_copy(out=o_sb, in_=ps)
                nc.sync.dma_start(
                    out=out[ds(g * QG + tt * P, P), ts(eg, DC)], in_=o_sb
                )

            # ---- pair state: est tiles + av/z banks, consumed one pair later
            class PairState:
                def __init__(self, g, hp):
                    self.g, self.hp = g, hp
                    self.ests = {}   # h -> list of 8 est tiles [P, 2, QG]
                    self.av = None   # [P, NQT, 2, DK] f32 psum (1 bank)
                    self.zt = None   # [P, QG] f32 psum (1 bank; cols 0:8 used)

            def attv_slice(ps_, s):
                """att@V + Z matmuls consuming est[s] (key tiles 2s, 2s+1)."""
                g, hp = ps_.g, ps_.hp
                if s == 0:
                    ps_.av = avp.tile(
                        [P, NQT, 2, DK], f32, tag="av", name=f"av_{g}_{hp}"
                    )
                    ps_.zt = zpool.tile([P, QG], f32, tag="z", name=f"z_{g}_{hp}")
                last = NKT // 2 - 1
                for kk in range(2):
                    kt = 2 * s + kk
                    for qt in range(NQT):
                        for hh in range(2):
                            h = 2 * hp + hh
                            est = ps_.ests[h][s]
                            stat = est[:, kk, ts(qt, P)]
                            first = s == 0 and kk == 0 and qt == 0 and hh == 0
                            lastm = s == last and kk == 1 and qt == NQT - 1 and hh == 1
                            nc.tensor.matmul(
                                ps_.av[:, qt, hh, :], stat,
                                v_aug[:, kt, h, 0:DK],
                                start=first, stop=lastm,
                            )
                            c = qt * 2 + hh
                            nc.tensor.matmul(
                                ps_.zt[:, c : c + 1], stat,
                                v_aug[:, kt, h, DK:VW],
                                start=first, stop=lastm,
                            )

            def finish_pair(ps_, o_sb_tiles, qts=None):
                """reciprocal + normalize for a finished pair.

                qts: restrict the normalize to these q-tiles (tail
                pipelining); reciprocal runs only when qts is None or
                starts at qt 0."""
                g, hp = ps_.g, ps_.hp
                if qts is None or qts[0] == 0:
                    ps_.rz = spool.tile(
                        [P, NQT, 2], f32r, tag="rz", name=f"rz_{g}_{hp}"
                    )
                    with nc.allow_low_precision("softmax denom reciprocal"):
                        nc.vector.reciprocal(
                            out=ps_.rz,
                            in_=ps_.zt[:, 0 : 2 * NQT].rearrange(
                                "p (q h) -> p q h", q=NQT
                            ),
                        )
                o_sb = o_sb_tiles[g]
                if qts is None:
                    nc.vector.tensor_tensor(
                        out=o_sb[:, :, 2 * hp : 2 * hp + 2, :],
                        in0=ps_.av,
                        in1=ps_.rz.unsqueeze(-1).broadcast_to([P, NQT, 2, DK]),
                        op=ALU.mult,
                    )
                else:
                    for qt in qts:
                        nc.vector.tensor_tensor(
                            out=o_sb[:, qt, 2 * hp : 2 * hp + 2, :],
                            in0=ps_.av[:, qt, :, :],
                            in1=ps_.rz[:, qt, :].unsqueeze(-1).broadcast_to(
                                [P, 2, DK]),
                            op=ALU.mult,
                        )

            def transposes(g, o_sb_tiles, attnT):
                o_sb = o_sb_tiles[g]
                for qt in range(NQT):
                    for fc in range(NFC):
                        nc.sync.dma_start_transpose(
                            out=attnT[:, fc, ts(qt, P)],
                            in_=o_sb[:, qt, 2 * fc : 2 * fc + 2, :],
                        )

            # =========== prelude ===========
            # DMA order tuned so the first-score chain (wk fc01, xk0, wq
            # fc01, xq0) clears in ~10us and fill-phase consumers (xv0, wv,
            # xk1-3) arrive before their spliced chains need them.
            wk_sb, wk_rest = load_w(wk, "w_k", fc_split=True, defer=True)
            xk_sbs = [load_x(xkT, 0, "x_k_0", tag="xk", bufs=4, split=True)]
            wq_sb, wq_rest = load_w(wq, "w_q", tag="wq", bufs=1, fc_split=True,
                                    defer=True)
            xq_tiles = {0: load_x(xqT, 0, "x_q_0", tag="xq", bufs=2, split=True)}
            kproj_chain(wk_sb, xk_sbs[0], 0, 0, half=0)
            kproj_chain(wk_sb, xk_sbs[0], 0, 0, half=1)

            qst = {0: qpool.tile([P, NFC, QG], bf16, tag="qT", name="qT_0")}
            qproj_chain(wq_sb, xq_tiles[0], qst[0], 0, 0, half=0)
            qproj_chain(wq_sb, xq_tiles[0], qst[0], 0, 0, half=1)

            xk_sbs.append(load_x(xkT, 1, "x_k_1", tag="xk", bufs=4))
            wv_sb = load_w(wv, "w_v")
            xv_tiles = {0: load_x(xvT, 0, "x_v_0", tag="xv", bufs=2)}
            xk_sbs.append(load_x(xkT, 2, "x_k_2", tag="xk", bufs=4))
            xk_sbs.append(load_x(xkT, 3, "x_k_3", tag="xk", bufs=4))
            wk_rest()
            wq_rest()
            nc.sync.dma_start(out=wo_sb, in_=wo.rearrange("(c p) e -> p c e", p=P))

            # =========== splice schedule ===========
            # pair index p = 4*g + hp runs score loop slots 0..7; sched[p][s]
            # is a list of thunks emitted before slot s's score matmuls.
            sched = {p: {s: [] for s in range(8)} for p in range(16)}

            def at(p, s, fn):
                sched[p][s].append(fn)

            # kproj: fc=0 for kg>=1 early in pair 0; fc=f in pair f-1... but
            # pair (0,hp) reads kT chunk hp for all kt: chunk fc must be fully
            # projected (all 4 kg) before pair (0,fc) starts.
            for kg, s_ in [(1, 0), (2, 1), (3, 3)]:
                at(0, s_, lambda kg=kg: kproj_chain(wk_sb, xk_sbs[kg], kg, 0))
            for fc in range(1, 4):
                for kg in range(4):
                    at(fc - 1, 2 * kg + 1, lambda kg=kg, fc=fc: kproj_chain(
                        wk_sb, xk_sbs[kg], kg, fc))
            # vproj: 10 tiles in pair 0 (extra on later slots), 6 in pair 1;
            # v_aug[kt] needed by attV(0,0) slice s=kt//2 at pair 1 slot s.
            # xv loads run >=2 slots ahead of their first vproj consumer.
            for vg, (p_, s_) in {1: (0, 1), 2: (0, 5), 3: (0, 7)}.items():
                at(p_, s_, lambda vg=vg: xv_tiles.__setitem__(
                    vg, load_x(xvT, vg, f"x_v_{vg}", tag="xv", bufs=2)))
            vq = [(0, 0, 1), (0, 1, 1), (0, 2, 1), (0, 3, 1), (0, 4, 2),
                  (0, 5, 2), (0, 6, 2), (0, 7, 2), (1, 0, 2), (1, 1, 2),
                  (1, 2, 2)]
            kt_next = 0
            for p_, s_, n_ in vq:
                for _ in range(n_):
                    if kt_next >= NKT:
                        break
                    kt = kt_next
                    kt_next += 1
                    at(p_, s_, lambda kt=kt: vproj_tile(
                        wv_sb, xv_tiles[kt // NQT], kt))
            # qproj for pair p+1 at pair p slot 5 (+ xq loads 2 pairs early)
            for p in range(15):
                g1, fc1 = divmod(p + 1, 4)
                if fc1 == 0 and g1 > 0:
                    at(p - 2 if p >= 2 else 0, 1, lambda g1=g1: xq_tiles.__setitem__(
                        g1, load_x(xqT, g1, f"x_q_{g1}", tag="xq", bufs=2)))
                    at(p, 5, lambda g1=g1: (
                        qst.__setitem__(g1, qpool.tile(
                            [P, NFC, QG], bf16, tag="qT", name=f"qT_{g1}")),
                        qproj_chain(wq_sb, xq_tiles[g1], qst[g1], g1, 0,
                                    half=0))[-1])
                    at(p, 7, lambda g1=g1: qproj_chain(
                        wq_sb, xq_tiles[g1], qst[g1], g1, 0, half=1))
                else:
                    at(p, 5, lambda g1=g1, fc1=fc1: qproj_chain(
                        wq_sb, xq_tiles[g1], qst[g1], g1, fc1, half=0))
                    at(p, 7, lambda g1=g1, fc1=fc1: qproj_chain(
                        wq_sb, xq_tiles[g1], qst[g1], g1, fc1, half=1))
            # outproj(g) chains spliced into pairs of group g+1
            op_slots = [(1, 4), (1, 6), (2, 2), (2, 4), (2, 6), (3, 2),
                        (3, 4), (3, 6)]
            attnT_holder = {}
            for g in range(3):
                for i, (hp_, s_) in enumerate(op_slots):
                    tt, eg = divmod(i, 2)
                    at(4 * (g + 1) + hp_, s_, lambda g=g, tt=tt, eg=eg: outproj_chain(
                        attnT_holder[g], g, tt, eg))

            # =========== main loop ===========
            o_sb_tiles = {}
            prev_pair = None   # PairState consumed by current pair's splices
            done_pair = None   # PairState whose attV completed last pair
            # (its finish_pair runs at the START of this pair so the DVE
            # queue never parks on unmet deps — DVE is in-order)

            for p in range(16):
                g, hp = divmod(p, 4)
                if g not in o_sb_tiles:
                    o_sb_tiles[g] = ospool.tile(
                        [P, NQT, HC, DK], bf16, tag="osb2", name=f"o_{g}"
                    )
                cur = PairState(g, hp)
                qT = qst[g]
                for kt2 in range(NKT // 2):
                    if kt2 == 0 and done_pair is not None:
                        finish_pair(done_pair, o_sb_tiles)
                        if done_pair.hp == NHP - 1:
                            gg = done_pair.g
                            attnT_holder[gg] = atpool.tile(
                                [P, NFC, QG], bf16, tag="attnT", name=f"aT_{gg}"
                            )
                            transposes(gg, o_sb_tiles, attnT_holder[gg])
                        done_pair = None
                    def emit_splices():
                        if prev_pair is not None:
                            attv_slice(prev_pair, kt2)
                        for fn in sched[p][kt2]:
                            fn()

                    def emit_scores():
                        sts = {}
                        for hh in range(2):
                            h = 2 * hp + hh
                            sts[h] = st_pool.tile(
                                [P, 2, QG], f32, tag="st",
                                name=f"st_{g}_{h}_{kt2}"
                            )
                        for kk in range(2):
                            kt = 2 * kt2 + kk
                            for hh in range(2):
                                h = 2 * hp + hh
                                r0 = hh * DK
                                nc.tensor.matmul(
                                    sts[h][:, kk, :],
                                    kT[r0 : r0 + DK, hp, ts(kt, P)],
                                    qT[r0 : r0 + DK, hp, :],
                                    start=True, stop=True,
                                    tile_position=(r0, 0),
                                )
                        for hh in range(2):
                            h = 2 * hp + hh
                            e = epool.tile(
                                [P, 2, QG], bf16, tag="est",
                                name=f"est_{g}_{h}_{kt2}"
                            )
                            cur.ests.setdefault(h, []).append(e)
                            nc.scalar.activation(
                                out=e, in_=sts[h], func=AF.Exp, scale=INV_SCALE
                            )

                    # fill phase (pairs 0-2): ACT is starved, so feed it
                    # scores before the heavy projection splices; steady
                    # state: splices first (PE uses the st-ring wait time)
                    if p < 3:
                        emit_scores()
                        emit_splices()
                    else:
                        emit_splices()
                        emit_scores()
                # previous pair's attV is complete; finish it at the start
                # of the next pair (deps met there, no DVE queue parking)
                done_pair = prev_pair
                prev_pair = cur

            # =========== tail: last pair's attV + outproj of group 3 ====
            # per-qt pipelining: as soon as qt's normalize lands, its
            # transposes, outproj chains and output DMA flow while the PE
            # works the next qt.
            finish_pair(done_pair, o_sb_tiles)
            for s in range(NKT // 2):
                attv_slice(prev_pair, s)
            attnT_holder[3] = atpool.tile(
                [P, NFC, QG], bf16, tag="attnT", name="aT_3"
            )
            o_sb3 = o_sb_tiles[3]
            for qt in range(NQT):
                finish_pair(prev_pair, o_sb_tiles, qts=[qt])
                for fc in range(NFC):
                    nc.sync.dma_start_transpose(
                        out=attnT_holder[3][:, fc, ts(qt, P)],
                        in_=o_sb3[:, qt, 2 * fc : 2 * fc + 2, :],
                    )
                for eg in range(2):
                    outproj_chain(attnT_holder[3], 3, qt, eg)

    nc.compile()
    return nc


def _get_nc(debug=False):
    if "nc" not in _CACHE:
        _CACHE["nc"] = _build()
    return _CACHE["nc"]


def _tf32(a):
    """Round fp32 to the TF32 grid (10-bit mantissa, round-to-nearest-even)."""
    u = np.ascontiguousarray(a, dtype=np.float32).view(np.uint32)
    u = (u + np.uint32(0xFFF) + ((u >> np.uint32(13)) & np.uint32(1))) & np.uint32(
        0xFFFFE000
    )
    return u.view(np.float32)


def _bf16(a):
    import ml_dtypes

    return np.ascontiguousarray(a, dtype=np.float32).astype(ml_dtypes.bfloat16)


def _make_in_maps(inputs):
    q = np.asarray(inputs["query"], dtype=np.float32)
    k = np.asarray(inputs["key"], dtype=np.float32)
    v = np.asarray(inputs["value"], dtype=np.float32)
    wq = np.asarray(inputs["wq"], dtype=np.float32)
    wk = np.asarray(inputs["wk"], dtype=np.float32)
    wv = np.asarray(inputs["wv"], dtype=np.float32)
    wo = np.asarray(inputs["wo"], dtype=np.float32)
    bq = np.asarray(inputs["bq"], dtype=np.float32)
    bk = np.asarray(inputs["bk"], dtype=np.float32)
    bv = np.asarray(inputs["bv"], dtype=np.float32)

    import ml_dtypes

    def _hl(a):
        hi = np.ascontiguousarray(a, dtype=np.float32).astype(
            ml_dtypes.float8_e4m3)
        lo = (a - hi.astype(np.float32)).astype(ml_dtypes.float8_e4m3)
        return hi, lo

    WS = 32.0  # fp8 weight pre-scale (undone via exp scale / ones column)
    xT = [(_hl(q[b].T), _hl(k[b].T), _hl(v[b].T)) for b in range(B)]
    in_maps = []
    for c in range(NCORES):
        b, g = divmod(c, 2)
        sl = slice(g * DC, (g + 1) * DC)
        wq8, wq8l = _hl(wq[:, sl] * WS)
        wk8, wk8l = _hl(wk[:, sl] * WS)
        wv8, wv8l = _hl(wv[:, sl] * WS)
        in_maps.append(
            {
                "xq8": xT[b][0][0], "xq8l": xT[b][0][1],
                "xk8": xT[b][1][0], "xk8l": xT[b][1][1],
                "xv8": xT[b][2][0], "xv8l": xT[b][2][1],
                "wq8": wq8, "wq8l": wq8l,
                "wk8": wk8, "wk8l": wk8l,
                "wv8": wv8, "wv8l": wv8l,
                "wo": _bf16(wo[sl, :]),
                "bq": np.ascontiguousarray(bq[sl] * WS),
                "bk": np.ascontiguousarray(bk[sl] * WS),
                "bv": np.ascontiguousarray(bv[sl] * WS),
            }
        )
    return in_maps


def run(inputs, **kwargs):
    """Run the kernel; returns (full_output, BassKernelResults)."""
    from concourse.bass_utils import run_bass_kernel_spmd

    kwargs.pop("debug", None)
    nc = _get_nc()
    in_maps = _make_in_maps(inputs)
    res = run_bass_kernel_spmd(nc, in_maps, core_ids=list(range(NCORES)), **kwargs)
    bo = np.asarray(inputs["bo"], dtype=np.float32)
    final = np.empty((B, S, D), np.float32)
    for b in range(B):
        final[b] = (
            res.results[2 * b]["out"].astype(np.float32)
            + res.results[2 * b + 1]["out"].astype(np.float32)
            + bo
        )
    return final, res


def kernel(**inputs):
    return run(inputs)[0]


# revision 14
# speedup vs baseline: 1.1516x; 1.0388x over previous
"""Multi-head attention (B=4, S=2048, D=1024, H=16) on 8 TRN2 NeuronCores.

Sharding (Megatron-style, per spec hint): data-parallel over batch (4) x
tensor-parallel over heads (2 groups of 8). Core c handles batch c//2,
head-group c%2. QKV projections column-sharded, output projection
row-sharded; the two partial outputs per batch are summed on the host
together with the output bias.

Per-core kernel (one NeuronCore, 8 heads, 2048 tokens), v2:
  - Scores transposed ST[k, q] with softmax-exp (no max subtraction) as one
    ACT pass per [128, 2, 512] score tile, bf16 out.
  - att@V uses the probabilities as the STATIONARY operand ([128k, 128q]
    slices) and v tiles [128k, 64] as moving, so the output [128q, 64]
    fills all 128 PSUM partitions: half the PE cost of the v-stationary
    form. A head-pair's whole output (4 q-tiles x 2 heads x 64) packs into
    exactly one PSUM bank with a single accumulation start/stop.
  - The softmax denominator Z accumulates via 1-column matmuls against the
    v_aug ones column into a separate z bank.
  - Normalration is one DVE pass per pair (stride-0 broadcast of 1/Z);
    the normalized [q, feature] tiles are transposed back to feature-major
    by the DMA xbar (dma_start_transpose), not the PE.
  - att@V chains are spliced into the NEXT pair's score loop (PE slack per
    kt2 slot), so ScalarE streams exps with few gaps; k/v/q projections and
    the previous group's output projection are spliced the same way.
"""

import sys

if "/opt/trn_rl_repo" not in sys.path:
    sys.path.insert(0, "/opt/trn_rl_repo")

import numpy as np

B, S, D = 4, 2048, 1024
H, DK = 16, 64
NCORES = 8
HC = H // 2            # heads per core
DC = HC * DK           # 512 local features per core
INV_SCALE = 1.0 / 8.0 / (32.0 * 32.0)  # 1/sqrt(DK), /32^2 fp8 weight scale
P = 128
NDCH = D // P          # 8 contraction chunks for projections
NFC = DC // P          # 4 local feature chunks
NKT = S // P           # 16 key tiles
NQG = 4                # query groups
QG = S // NQG          # 512 queries per group
NQT = QG // P          # 4 query tiles per group
VW = DK + 1            # 65: v columns + ones column
NHP = HC // 2          # head pairs

_CACHE = {}


def _build():
    import concourse.bass as bass
    import concourse.bacc as bacc
    import concourse.tile as tile
    import concourse.mybir as mybir
    from concourse.bass import ts, ds

    f32 = mybir.dt.float32
    f32r = mybir.dt.float32r
    bf16 = mybir.dt.bfloat16
    AF = mybir.ActivationFunctionType
    ALU = mybir.AluOpType

    LOG2E = 1.4426950408889634
    SCH_A = INV_SCALE * LOG2E * 128.0
    SCH_B = 16256.0 - 5.5 + 0.5  # centering + trunc->round bias

    nc = bacc.Bacc("TRN2", target_bir_lowering=False, num_devices=NCORES)

    f8 = mybir.dt.float8e4
    DR = mybir.MatmulPerfMode.DoubleRow
    xqT = (nc.dram_tensor("xq8", [D, S], f8, kind="ExternalInput"),
           nc.dram_tensor("xq8l", [D, S], f8, kind="ExternalInput"))
    xkT = (nc.dram_tensor("xk8", [D, S], f8, kind="ExternalInput"),
           nc.dram_tensor("xk8l", [D, S], f8, kind="ExternalInput"))
    xvT = (nc.dram_tensor("xv8", [D, S], f8, kind="ExternalInput"),
           nc.dram_tensor("xv8l", [D, S], f8, kind="ExternalInput"))
    wq = (nc.dram_tensor("wq8", [D, DC], f8, kind="ExternalInput"),
          nc.dram_tensor("wq8l", [D, DC], f8, kind="ExternalInput"))
    wk = (nc.dram_tensor("wk8", [D, DC], f8, kind="ExternalInput"),
          nc.dram_tensor("wk8l", [D, DC], f8, kind="ExternalInput"))
    wv = (nc.dram_tensor("wv8", [D, DC], f8, kind="ExternalInput"),
          nc.dram_tensor("wv8l", [D, DC], f8, kind="ExternalInput"))
    wo = nc.dram_tensor("wo", [DC, D], bf16, kind="ExternalInput")
    bq = nc.dram_tensor("bq", [DC], f32, kind="ExternalInput")
    bk = nc.dram_tensor("bk", [DC], f32, kind="ExternalInput")
    bv = nc.dram_tensor("bv", [DC], f32, kind="ExternalInput")
    out = nc.dram_tensor("out", [S, D], bf16, kind="ExternalOutput")

    with tile.TileContext(nc) as tc:
        with (
            tc.tile_pool(name="persist", bufs=1) as persist,
            tc.tile_pool(name="wts", bufs=2) as wpool,
            tc.tile_pool(name="xin", bufs=4) as xpool,
            tc.tile_pool(name="qt", bufs=2) as qpool,
            tc.tile_pool(name="expst", bufs=18) as epool,
            tc.tile_pool(name="osb", bufs=2) as ospool,
            tc.tile_pool(name="att", bufs=1) as atpool,
            tc.tile_pool(name="small", bufs=2) as spool,
            tc.tile_pool(name="oc", bufs=2) as ocpool,
            tc.tile_pool(name="pp", bufs=2, space="PSUM") as pp,
            tc.tile_pool(name="st", bufs=2, space="PSUM") as st_pool,
            tc.tile_pool(name="av", bufs=1, space="PSUM") as avp,
            tc.tile_pool(name="zp", bufs=1, space="PSUM") as zpool,
        ):
            # ---- persistent SBUF tensors ----
            kT = persist.tile([P, NFC, S], bf16)          # 16KB/part
            v_aug = persist.tile([P, NKT, HC, VW], bf16)  # ~16.6KB/part
            wo_sb = persist.tile([P, NFC, D], bf16)       # 8KB/part
            bq_sb = persist.tile([P, NFC], f32)
            bk_sb = persist.tile([P, NFC], f32)
            bvb = persist.tile([P, DC], f32)              # bias_v broadcast

            nc.sync.dma_start(out=bq_sb, in_=bq.rearrange("(c p) -> p c", p=P))
            nc.sync.dma_start(out=bk_sb, in_=bk.rearrange("(c p) -> p c", p=P))
            bv_ap = bv.ap()
            bvb_src = bass.AP(
                tensor=bv_ap.tensor, offset=bv_ap.offset, ap=[[0, P], *bv_ap.ap]
            )
            nc.sync.dma_start(out=bvb, in_=bvb_src)
            # ones column of v_aug (softmax denominator trick)
            ones_st = persist.tile([P, P], f32)
            nc.vector.memset(ones_st, 32.0)
            nc.vector.tensor_copy(
                out=v_aug[:, :, :, DK],
                in_=ones_st.rearrange("p (k h) -> p k h", k=NKT),
            )

            # ---- emission helpers (PE program order == emission order) ----
            def load_w(w_dram, name, tag="w", bufs=None, fc_split=False,
                       defer=False):
                pair = []
                for i, wd in enumerate(w_dram):
                    pair.append(wpool.tile(
                        [P, NDCH, DC], f8, tag=f"{tag}{i}", name=f"{name}_{i}",
                        bufs=bufs))
                rests = []
                for w_sb, wd in zip(pair, w_dram):
                    wr = wd.rearrange("(c p) f -> p c f", p=P)
                    if fc_split:
                        nc.sync.dma_start(
                            out=w_sb[:, :, 0:DC // 2], in_=wr[:, :, 0:DC // 2])
                        rests.append(lambda w_sb=w_sb, wr=wr: nc.sync.dma_start(
                            out=w_sb[:, :, DC // 2:], in_=wr[:, :, DC // 2:]))
                    else:
                        nc.sync.dma_start(out=w_sb, in_=wr)
                if fc_split:
                    rest = lambda: [r() for r in rests]
                    if defer:
                        return tuple(pair), rest
                    rest()
                return tuple(pair)

            def load_x(xT_dram, g, name, tag="x", bufs=None, split=False):
                pair = []
                for i, xd in enumerate(xT_dram):
                    x_sb = xpool.tile(
                        [P, NDCH, QG], f8, tag=f"{tag}{i}", name=f"{name}_{i}",
                        bufs=bufs)
                    pair.append(x_sb)
                    xr = xd.rearrange("(c p) t -> p c t", p=P)[:, :, ts(g, QG)]
                    if split:
                        h_ = NDCH // 2
                        nc.sync.dma_start(out=x_sb[:, 0:h_, :], in_=xr[:, 0:h_, :])
                        nc.sync.dma_start(out=x_sb[:, h_:, :], in_=xr[:, h_:, :])
                    else:
                        nc.sync.dma_start(out=x_sb, in_=xr)
                return tuple(pair)

            def proj_mms(ps, w_pair, x_pair, fc, half):
                """3-term hi/lo fp8 DoubleRow chain: w8*x8 + w8*x8l + w8l*x8.
                Contraction pairs c of 256 rows; 3 DR matmuls each."""
                w8, w8l = w_pair
                x8, x8l = x_pair
                cs = range(0, NDCH // 4) if half == 0 else (
                    range(NDCH // 4, NDCH // 2) if half == 1
                    else range(NDCH // 2))
                ncp = NDCH // 2
                for c in cs:
                    d = slice(2 * c, 2 * c + 2)
                    for t, (wt, xt) in enumerate(
                        ((w8, x8), (w8, x8l), (w8l, x8))
                    ):
                        nc.tensor.matmul(
                            ps, wt[:, d, ts(fc, P)], xt[:, d, :],
                            start=(c == 0 and t == 0),
                            stop=(c == ncp - 1 and t == 2),
                            perf_mode=DR,
                        )

            def kproj_chain(w_sb, x_sb, g, fc, half=None, state={}):
                if half in (None, 0):
                    state["ps"] = pp.tile(
                        [P, QG], f32, tag="pp", name=f"pk_{g}_{fc}"
                    )
                ps = state["ps"]
                proj_mms(ps, w_sb, x_sb, fc, half)
                if half in (None, 1):
                    nc.vector.tensor_scalar(
                        out=kT[:, fc, ts(g, QG)], in0=ps,
                        scalar1=bk_sb[:, fc : fc + 1], scalar2=None, op0=ALU.add,
                    )

            def qproj_chain(w_sb, x_sb, qT, g, fc, half=None, state={}):
                if half in (None, 0):
                    state["ps"] = pp.tile(
                        [P, QG], f32, tag="pp", name=f"pq_{g}_{fc}"
                    )
                ps = state["ps"]
                proj_mms(ps, w_sb, x_sb, fc, half)
                if half in (None, 1):
                    nc.vector.tensor_scalar(
                        out=qT[:, fc, :], in0=ps,
                        scalar1=bq_sb[:, fc : fc + 1], scalar2=None, op0=ALU.add,
                    )

            def vproj_tile(w_sb, x_sb, kt):
                tt = kt % NQT
                w8, w8l = w_sb
                x8, x8l = x_sb
                ps = pp.tile([P, DC], f32, tag="pp", name=f"pv_{kt}")
                ncp = NDCH // 2
                for c in range(ncp):
                    d = slice(2 * c, 2 * c + 2)
                    for t, (xt, wt) in enumerate(
                        ((x8, w8), (x8, w8l), (x8l, w8))
                    ):
                        nc.tensor.matmul(
                            ps, xt[:, d, ts(tt, P)], wt[:, d, :],
                            start=(c == 0 and t == 0),
                            stop=(c == ncp - 1 and t == 2),
                            perf_mode=DR,
                        )
                nc.vector.tensor_add(
                    out=v_aug[:, kt, :, 0:DK],
                    in0=ps.rearrange("p (h d) -> p h d", h=HC),
                    in1=bvb.rearrange("p (h d) -> p h d", h=HC),
                )

            def outproj_chain(attnT, g, tt, eg, pool=None):
                pool = pool or pp
                ps = pool.tile(
                    [P, DC], f32, tag="pp" if pool is pp else "av",
                    name=f"po_{g}_{tt}_{eg}",
                )
                for fc in range(NFC):
                    nc.tensor.matmul(
                        ps, attnT[:, fc, ts(tt, P)], wo_sb[:, fc, ts(eg, DC)],
                        start=(fc == 0), stop=(fc == NFC - 1),
                    )
                o_sb = ocpool.tile([P, DC], bf16, tag="osb", name=f"ob_{g}_{tt}_{eg}")
                nc.vector.tensor_copy(out=o_sb, in_=ps)
                nc.sync.dma_start(
                    out=out[ds(g * QG + tt * P, P), ts(eg, DC)], in_=o_sb
                )

            # ---- pair state: est tiles + av/z banks, consumed one pair later
            class PairState:
                def __init__(self, g, hp):
                    self.g, self.hp = g, hp
                    self.ests = {}   # h -> list of 8 est tiles [P, 2, QG]
                    self.av = None   # [P, NQT, 2, DK] f32 psum (1 bank)
                    self.zt = None   # [P, QG] f32 psum (1 bank; cols 0:8 used)

            def attv_slice(ps_, s):
                """att@V + Z matmuls consuming est[s] (key tiles 2s, 2s+1)."""
                g, hp = ps_.g, ps_.hp
                if s == 0:
                    ps_.av = avp.tile(
                        [P, NQT, 2, DK], f32, tag="av", name=f"av_{g}_{hp}"
                    )
                    ps_.zt = zpool.tile([P, QG], f32, tag="z", name=f"z_{g}_{hp}")
                last = NKT // 2 - 1
                for kk in range(2):
                    kt = 2 * s + kk
                    for qt in range(NQT):
                        for hh in range(2):
                            h = 2 * hp + hh
                            est = ps_.ests[h][s]
                            stat = est[:, kk, ts(qt, P)]
                            first = s == 0 and kk == 0 and qt == 0 and hh == 0
                            lastm = s == last and kk == 1 and qt == NQT - 1 and hh == 1
                            nc.tensor.matmul(
                                ps_.av[:, qt, hh, :], stat,
                                v_aug[:, kt, h, 0:DK],
                                start=first, stop=lastm,
                            )
                            c = qt * 2 + hh
                            nc.tensor.matmul(
                                ps_.zt[:, c : c + 1], stat,
                                v_aug[:, kt, h, DK:VW],
                                start=first, stop=lastm,
                            )

            def finish_pair(ps_, o_sb_tiles, qts=None):
                """reciprocal + normalize for a finished pair.

                qts: restrict the normalize to these q-tiles (tail
                pipelining); reciprocal runs only when qts is None or
                starts at qt 0."""
                g, hp = ps_.g, ps_.hp
                if qts is None or qts[0] == 0:
                    ps_.rz = spool.tile(
                        [P, NQT, 2], f32r, tag="rz", name=f"rz_{g}_{hp}"
                    )
                    with nc.allow_low_precision("softmax denom reciprocal"):
                        nc.vector.reciprocal(
                            out=ps_.rz,
                            in_=ps_.zt[:, 0 : 2 * NQT].rearrange(
                                "p (q h) -> p q h", q=NQT
                            ),
                        )
                o_sb = o_sb_tiles[g]
                if qts is None:
                    nc.vector.tensor_tensor(
                        out=o_sb[:, :, 2 * hp : 2 * hp + 2, :],
                        in0=ps_.av,
                        in1=ps_.rz.unsqueeze(-1).broadcast_to([P, NQT, 2, DK]),
                        op=ALU.mult,
                    )
                else:
                    for qt in qts:
                        nc.vector.tensor_tensor(
                            out=o_sb[:, qt, 2 * hp : 2 * hp + 2, :],
                            in0=ps_.av[:, qt, :, :],
                            in1=ps_.rz[:, qt, :].unsqueeze(-1).broadcast_to(
                                [P, 2, DK]),
                            op=ALU.mult,
                        )

            def transposes(g, o_sb_tiles, attnT):
                o_sb = o_sb_tiles[g]
                for qt in range(NQT):
                    for fc in range(NFC):
                        nc.sync.dma_start_transpose(
                            out=attnT[:, fc, ts(qt, P)],
                            in_=o_sb[:, qt, 2 * fc : 2 * fc + 2, :],
                        )

            # =========== prelude ===========
            # DMA order tuned so the first-score chain (wk fc01, xk0, wq
            # fc01, xq0) clears in ~10us and fill-phase consumers (xv0, wv,
            # xk1-3) arrive before their spliced chains need them.
            wk_sb, wk_rest = load_w(wk, "w_k", fc_split=True, defer=True)
            xk_sbs = [load_x(xkT, 0, "x_k_0", tag="xk", bufs=4, split=True)]
            wq_sb, wq_rest = load_w(wq, "w_q", tag="wq", bufs=1, fc_split=True,
                                    defer=True)
            xq_tiles = {0: load_x(xqT, 0, "x_q_0", tag="xq", bufs=2, split=True)}
            kproj_chain(wk_sb, xk_sbs[0], 0, 0, half=0)
            kproj_chain(wk_sb, xk_sbs[0], 0, 0, half=1)

            qst = {0: qpool.tile([P, NFC, QG], bf16, tag="qT", name="qT_0")}
            qproj_chain(wq_sb, xq_tiles[0], qst[0], 0, 0, half=0)
            qproj_chain(wq_sb, xq_tiles[0], qst[0], 0, 0, half=1)

            xk_sbs.append(load_x(xkT, 1, "x_k_1", tag="xk", bufs=4))
            wv_sb = load_w(wv, "w_v")
            xv_tiles = {0: load_x(xvT, 0, "x_v_0", tag="xv", bufs=2)}
            xk_sbs.append(load_x(xkT, 2, "x_k_2", tag="xk", bufs=4))
            xk_sbs.append(load_x(xkT, 3, "x_k_3", tag="xk", bufs=4))
            wk_rest()
            wq_rest()
            nc.sync.dma_start(out=wo_sb, in_=wo.rearrange("(c p) e -> p c e", p=P))

            # =========== splice schedule ===========
            # pair index p = 4*g + hp runs score loop slots 0..7; sched[p][s]
            # is a list of thunks emitted before slot s's score matmuls.
            sched = {p: {s: [] for s in range(8)} for p in range(16)}

            def at(p, s, fn):
                sched[p][s].append(fn)

            # kproj: fc=0 for kg>=1 early in pair 0; fc=f in pair f-1... but
            # pair (0,hp) reads kT chunk hp for all kt: chunk fc must be fully
            # projected (all 4 kg) before pair (0,fc) starts.
            for kg, s_ in [(1, 0), (2, 1), (3, 3)]:
                at(0, s_, lambda kg=kg: kproj_chain(wk_sb, xk_sbs[kg], kg, 0))
            for fc in range(1, 4):
                for kg in range(4):
                    at(fc - 1, 2 * kg + 1, lambda kg=kg, fc=fc: kproj_chain(
                        wk_sb, xk_sbs[kg], kg, fc))
            # vproj: 10 tiles in pair 0 (extra on later slots), 6 in pair 1;
            # v_aug[kt] needed by attV(0,0) slice s=kt//2 at pair 1 slot s.
            # xv loads run >=2 slots ahead of their first vproj consumer.
            for vg, (p_, s_) in {1: (0, 1), 2: (0, 5), 3: (0, 7)}.items():
                at(p_, s_, lambda vg=vg: xv_tiles.__setitem__(
                    vg, load_x(xvT, vg, f"x_v_{vg}", tag="xv", bufs=2)))
            vq = [(0, 0, 1), (0, 1, 1), (0, 2, 1), (0, 3, 1), (0, 4, 2),
                  (0, 5, 2), (0, 6, 2), (0, 7, 2), (1, 0, 2), (1, 1, 2),
                  (1, 2, 2)]
            kt_next = 0
            for p_, s_, n_ in vq:
                for _ in range(n_):
                    if kt_next >= NKT:
                        break
                    kt = kt_next
                    kt_next += 1
                    at(p_, s_, lambda kt=kt: vproj_tile(
                        wv_sb, xv_tiles[kt // NQT], kt))
            # qproj for pair p+1 at pair p slot 5 (+ xq loads 2 pairs early)
            for p in range(15):
                g1, fc1 = divmod(p + 1, 4)
                if fc1 == 0 and g1 > 0:
                    at(p - 2 if p >= 2 else 0, 1, lambda g1=g1: xq_tiles.__setitem__(
                        g1, load_x(xqT, g1, f"x_q_{g1}", tag="xq", bufs=2)))
                    at(p, 5, lambda g1=g1: (
                        qst.__setitem__(g1, qpool.tile(
                            [P, NFC, QG], bf16, tag="qT", name=f"qT_{g1}")),
                        qproj_chain(wq_sb, xq_tiles[g1], qst[g1], g1, 0,
                                    half=0))[-1])
                    at(p, 7, lambda g1=g1: qproj_chain(
                        wq_sb, xq_tiles[g1], qst[g1], g1, 0, half=1))
                else:
                    at(p, 5, lambda g1=g1, fc1=fc1: qproj_chain(
                        wq_sb, xq_tiles[g1], qst[g1], g1, fc1, half=0))
                    at(p, 7, lambda g1=g1, fc1=fc1: qproj_chain(
                        wq_sb, xq_tiles[g1], qst[g1], g1, fc1, half=1))
            # outproj(g) chains spliced into pairs of group g+1
            op_slots = [(1, 4), (1, 6), (2, 2), (2, 4), (2, 6), (3, 2),
                        (3, 4), (3, 6)]
            attnT_holder = {}
            for g in range(3):
                for i, (hp_, s_) in enumerate(op_slots):
                    tt, eg = divmod(i, 2)
                    at(4 * (g + 1) + hp_, s_, lambda g=g, tt=tt, eg=eg: outproj_chain(
                        attnT_holder[g], g, tt, eg))

            # =========== main loop ===========
            o_sb_tiles = {}
            prev_pair = None   # PairState consumed by current pair's splices
            done_pair = None   # PairState whose attV completed last pair
            # (its finish_pair runs at the START of this pair so the DVE
            # queue never parks on unmet deps — DVE is in-order)

            for p in range(16):
                g, hp = divmod(p, 4)
                if g not in o_sb_tiles:
                    o_sb_tiles[g] = ospool.tile(
                        [P, NQT, HC, DK], bf16, tag="osb2", name=f"o_{g}"
                    )
                cur = PairState(g, hp)
                qT = qst[g]
                for kt2 in range(NKT // 2):
                    if kt2 == 0 and done_pair is not None:
                        finish_pair(done_pair, o_sb_tiles)
                        if done_pair.hp == NHP - 1:
                            gg = done_pair.g
                            attnT_holder[gg] = atpool.tile(
                                [P, NFC, QG], bf16, tag="attnT", name=f"aT_{gg}"
                            )
                            transposes(gg, o_sb_tiles, attnT_holder[gg])
                        done_pair = None
                    def emit_splices():
                        if prev_pair is not None:
                            attv_slice(prev_pair, kt2)
                        for fn in sched[p][kt2]:
                            fn()

                    def emit_scores():
                        sts = {}
                        for hh in range(2):
                            h = 2 * hp + hh
                            sts[h] = st_pool.tile(
                                [P, 2, QG], f32, tag="st",
                                name=f"st_{g}_{h}_{kt2}"
                            )
                        for kk in range(2):
                            kt = 2 * kt2 + kk
                            for hh in range(2):
                                h = 2 * hp + hh
                                r0 = hh * DK
                                nc.tensor.matmul(
                                    sts[h][:, kk, :],
                                    kT[r0 : r0 + DK, hp, ts(kt, P)],
                                    qT[r0 : r0 + DK, hp, :],
                                    start=True, stop=True,
                                    tile_position=(r0, 0),
                                )
                        for hh in range(2):
                            h = 2 * hp + hh
                            e = epool.tile(
                                [P, 2, QG], bf16, tag="est",
                                name=f"est_{g}_{h}_{kt2}"
                            )
                            cur.ests.setdefault(h, []).append(e)
                            if hh == 1 and kt2 % 2 == 1:
                                # Schraudolph exp on DVE: bf16 bit pattern of
                                # exp(s*INV_SCALE) ~= trunc(A*s + B); offloads
                                # 25% of the exp stream from ScalarE (max rel
                                # err ~3%, partially cancelled by softmax)
                                nc.vector.tensor_scalar(
                                    out=e.bitcast(mybir.dt.int16),
                                    in0=sts[h],
                                    scalar1=SCH_A, scalar2=SCH_B,
                                    op0=ALU.mult, op1=ALU.add,
                                )
                            else:
                                nc.scalar.activation(
                                    out=e, in_=sts[h], func=AF.Exp,
                                    scale=INV_SCALE
                                )

                    # fill phase (pairs 0-2): ACT is starved, so feed it
                    # scores before the heavy projection splices; steady
                    # state: splices first (PE uses the st-ring wait time)
                    if p < 3:
                        emit_scores()
                        emit_splices()
                    else:
                        emit_splices()
                        emit_scores()
                # previous pair's attV is complete; finish it at the start
                # of the next pair (deps met there, no DVE queue parking)
                done_pair = prev_pair
                prev_pair = cur

            # =========== tail: last pair's attV + outproj of group 3 ====
            # per-qt pipelining: as soon as qt's normalize lands, its
            # transposes, outproj chains and output DMA flow while the PE
            # works the next qt.
            finish_pair(done_pair, o_sb_tiles)
            for s in range(NKT // 2):
                attv_slice(prev_pair, s)
            attnT_holder[3] = atpool.tile(
                [P, NFC, QG], bf16, tag="attnT", name="aT_3"
            )
            o_sb3 = o_sb_tiles[3]
            for qt in range(NQT):
                finish_pair(prev_pair, o_sb_tiles, qts=[qt])
                for fc in range(NFC):
                    nc.sync.dma_start_transpose(
                        out=attnT_holder[3][:, fc, ts(qt, P)],
                        in_=o_sb3[:, qt, 2 * fc : 2 * fc + 2, :],
                    )
                for eg in range(2):
                    outproj_chain(attnT_holder[3], 3, qt, eg)

    nc.compile()
    return nc


def _get_nc(debug=False):
    if "nc" not in _CACHE:
        _CACHE["nc"] = _build()
    return _CACHE["nc"]


def _tf32(a):
    """Round fp32 to the TF32 grid (10-bit mantissa, round-to-nearest-even)."""
    u = np.ascontiguousarray(a, dtype=np.float32).view(np.uint32)
    u = (u + np.uint32(0xFFF) + ((u >> np.uint32(13)) & np.uint32(1))) & np.uint32(
        0xFFFFE000
    )
    return u.view(np.float32)


def _bf16(a):
    import ml_dtypes

    return np.ascontiguousarray(a, dtype=np.float32).astype(ml_dtypes.bfloat16)


def _make_in_maps(inputs):
    q = np.asarray(inputs["query"], dtype=np.float32)
    k = np.asarray(inputs["key"], dtype=np.float32)
    v = np.asarray(inputs["value"], dtype=np.float32)
    wq = np.asarray(inputs["wq"], dtype=np.float32)
    wk = np.asarray(inputs["wk"], dtype=np.float32)
    wv = np.asarray(inputs["wv"], dtype=np.float32)
    wo = np.asarray(inputs["wo"], dtype=np.float32)
    bq = np.asarray(inputs["bq"], dtype=np.float32)
    bk = np.asarray(inputs["bk"], dtype=np.float32)
    bv = np.asarray(inputs["bv"], dtype=np.float32)

    import ml_dtypes

    def _hl(a):
        hi = np.ascontiguousarray(a, dtype=np.float32).astype(
            ml_dtypes.float8_e4m3)
        lo = (a - hi.astype(np.float32)).astype(ml_dtypes.float8_e4m3)
        return hi, lo

    WS = 32.0  # fp8 weight pre-scale (undone via exp scale / ones column)
    xT = [(_hl(q[b].T), _hl(k[b].T), _hl(v[b].T)) for b in range(B)]
    in_maps = []
    for c in range(NCORES):
        b, g = divmod(c, 2)
        sl = slice(g * DC, (g + 1) * DC)
        wq8, wq8l = _hl(wq[:, sl] * WS)
        wk8, wk8l = _hl(wk[:, sl] * WS)
        wv8, wv8l = _hl(wv[:, sl] * WS)
        in_maps.append(
            {
                "xq8": xT[b][0][0], "xq8l": xT[b][0][1],
                "xk8": xT[b][1][0], "xk8l": xT[b][1][1],
                "xv8": xT[b][2][0], "xv8l": xT[b][2][1],
                "wq8": wq8, "wq8l": wq8l,
                "wk8": wk8, "wk8l": wk8l,
                "wv8": wv8, "wv8l": wv8l,
                "wo": _bf16(wo[sl, :]),
                "bq": np.ascontiguousarray(bq[sl] * WS),
                "bk": np.ascontiguousarray(bk[sl] * WS),
                "bv": np.ascontiguousarray(bv[sl] * WS),
            }
        )
    return in_maps


def run(inputs, **kwargs):
    """Run the kernel; returns (full_output, BassKernelResults)."""
    from concourse.bass_utils import run_bass_kernel_spmd

    kwargs.pop("debug", None)
    nc = _get_nc()
    in_maps = _make_in_maps(inputs)
    res = run_bass_kernel_spmd(nc, in_maps, core_ids=list(range(NCORES)), **kwargs)
    bo = np.asarray(inputs["bo"], dtype=np.float32)
    final = np.empty((B, S, D), np.float32)
    for b in range(B):
        final[b] = (
            res.results[2 * b]["out"].astype(np.float32)
            + res.results[2 * b + 1]["out"].astype(np.float32)
            + bo
        )
    return final, res


def kernel(**inputs):
    return run(inputs)[0]
